# revision 8
# baseline (speedup 1.0000x reference)
"""Causal self-attention (B=4, T=2048, C=1024, H=16, D=64) on 8 trn2 cores.

Sharding: core c handles batch b = c//2 and head-group hg = c%2 (8 heads).
qkv projection is column-parallel, attention is head-parallel, out_proj is
row-parallel; the final 2-way partial-sum + bias happens on host.

Per-core device program (identical SPMD program, different data):
  phase 1: qkvT = (W_slice.T @ x.T) + bias   -> qT,kT [feat, tok], vT [feat, tok]
  phase 2: v2   = transpose(vT) interleaved with ones columns (softmax denom trick)
  phase 3: per (head, q-chunk): scoresT = kT.T@qT (causal-skipped), exp on ACT,
           causal mask on DVE, ctxT_ext = [v|1].T @ exp accumulated in PSUM
           (row 64 of ctxT_ext = softmax denominator, for free)
  phase 4: batched reciprocal of all denominators, broadcast via K=32 selector
           matmul, normalize ctx in place
  phase 5: y_partial = ctx_stacked.T @ W_out_slice -> DRAM
"""

import sys

if "/opt/trn_rl_repo" not in sys.path:
    sys.path.insert(0, "/opt/trn_rl_repo")

import numpy as np

B, T, C = 4, 2048, 1024
H, D = 16, 64
NCORES = 8
HPC = 8          # heads per core
FQ = HPC * D     # 512 per-core q (=k=v) feature count
TK = T // 128    # 16 token tiles of 128
QC = T // 512    # 4 q-chunks of 512
V2W = 130        # v2 per-ktile width: 64 + 1 + 64 + 1

_CACHE = {}


def _build_program():
    import concourse.bacc as bacc
    import concourse.tile as tile
    import concourse.mybir as mybir
    from concourse.masks import make_identity
    from contextlib import ExitStack

    f32 = mybir.dt.float32
    AF = mybir.ActivationFunctionType

    nc = bacc.Bacc("TRN2", target_bir_lowering=False, debug=False)

    x_t = nc.dram_tensor("x_t", [C, T], f32, kind="ExternalInput").ap()
    w_s = nc.dram_tensor("w_s", [C, 3 * FQ], f32, kind="ExternalInput").ap()
    b_s = nc.dram_tensor("b_s", [3 * FQ], f32, kind="ExternalInput").ap()
    w_o = nc.dram_tensor("w_o", [FQ, C], f32, kind="ExternalInput").ap()
    mask_d = nc.dram_tensor("mask", [128, 2048], f32, kind="ExternalInput").ap()
    sel_d = nc.dram_tensor("sel", [32, 2048], f32, kind="ExternalInput").ap()
    y_d = nc.dram_tensor("y", [T, C], f32, kind="ExternalOutput").ap()

    with tile.TileContext(nc) as tc, ExitStack() as ctx:
        # ---- pools by lifetime ----
        # whole-kernel persistents
        pp = ctx.enter_context(tc.tile_pool(name="persist", bufs=1))
        ident = pp.tile([128, 128], f32, tag="ident", name="ident")
        make_identity(nc, ident)
        b_sb = pp.tile([128, 12], f32, tag="bias", name="b_sb")
        nc.sync.dma_start(out=b_sb, in_=b_s.rearrange("(f p) -> p f", p=128))
        coll = pp.tile([32, 512], f32, tag="coll", name="coll")
        collr = pp.tile([32, 512], f32, tag="collr", name="collr")

        # q/k live phases 1-3; v2 lives 2-3; vT lives 1-2; ctx lives 3-5
        qk_pool = ctx.enter_context(tc.tile_pool(name="qk", bufs=1))
        q_sb = [qk_pool.tile([128, T], f32, tag=f"q{i}", name=f"q_sb{i}") for i in range(4)]
        k_sb = [qk_pool.tile([128, T], f32, tag=f"k{i}", name=f"k_sb{i}") for i in range(4)]
        v2_pool = ctx.enter_context(tc.tile_pool(name="v2p", bufs=1))
        v2_sb = v2_pool.tile([128, 4 * TK * V2W], f32, tag="v2", name="v2_sb")
        nc.gpsimd.memset(v2_sb, 1.0)

        # ---------------- phase 1: qkv projection ----------------
        with tc.tile_pool(name="xres", bufs=1) as xp, \
             tc.tile_pool(name="wstr", bufs=2) as wp, \
             tc.tile_pool(name="vTp", bufs=1) as vTp, \
             tc.tile_pool(name="ps1", bufs=2, space="PSUM") as ps1, \
             tc.tile_pool(name="pst", bufs=4, space="PSUM") as pst:
            x_sb = [xp.tile([128, T], f32, tag=f"x{ks}", name=f"x_sb{ks}") for ks in range(8)]
            for ks in range(8):
                nc.sync.dma_start(out=x_sb[ks], in_=x_t[ks * 128:(ks + 1) * 128, :])
            vT_sb = [vTp.tile([128, T], f32, tag=f"vT{i}", name=f"vT_sb{i}") for i in range(4)]

            for ft in range(12):
                wt = [wp.tile([128, 128], f32, tag=f"w{ks}", name=f"wt{ks}") for ks in range(8)]
                for ks in range(8):
                    nc.sync.dma_start(
                        out=wt[ks],
                        in_=w_s[ks * 128:(ks + 1) * 128, ft * 128:(ft + 1) * 128])
                if ft < 4:
                    dest = q_sb[ft]
                elif ft < 8:
                    dest = k_sb[ft - 4]
                else:
                    dest = vT_sb[ft - 8]
                for tcn in range(4):
                    ps = ps1.tile([128, 512], f32, tag="qkvps", name="qkv_ps")
                    for ks in range(8):
                        nc.tensor.matmul(
                            ps, lhsT=wt[ks],
                            rhs=x_sb[ks][:, tcn * 512:(tcn + 1) * 512],
                            start=(ks == 0), stop=(ks == 7))
                    # evacuate with fused bias add: out = 1.0*psum + b
                    nc.scalar.activation(
                        dest[:, tcn * 512:(tcn + 1) * 512], ps,
                        AF.Identity, bias=b_sb[:, ft:ft + 1], scale=1.0)

            # ---------------- phase 2: build v2 (transposed v + ones cols) ----
            for pr in range(4):
                for kt in range(TK):
                    pt = pst.tile([128, 128], f32, tag="tp", name="tr_ps")
                    nc.tensor.transpose(
                        pt, vT_sb[pr][:, kt * 128:(kt + 1) * 128], ident)
                    base = (pr * TK + kt) * V2W
                    nc.vector.tensor_copy(v2_sb[:, base:base + 64], pt[:, 0:64])
                    nc.vector.tensor_copy(
                        v2_sb[:, base + 65:base + 129], pt[:, 64:128])

        # ---------------- phase 3: attention ----------------
        # ctx + mask allocated now that the phase-1 pools are closed
        ctx_pool = ctx.enter_context(tc.tile_pool(name="ctxp", bufs=1))
        ctx_sb = [ctx_pool.tile([128, T], f32, tag=f"c{i}", name=f"ctx_sb{i}") for i in range(4)]
        mask_pool = ctx.enter_context(tc.tile_pool(name="maskp", bufs=1))
        mask_sb = mask_pool.tile([128, 2048], f32, tag="mask", name="mask_sb")
        nc.sync.dma_start(out=mask_sb, in_=mask_d)
        with tc.tile_pool(name="scps", bufs=1, space="PSUM") as scps, \
             tc.tile_pool(name="cxps", bufs=2, space="PSUM") as cxps, \
             tc.tile_pool(name="stg", bufs=4) as stg, \
             tc.tile_pool(name="esb", bufs=3) as esb:
            for h in range(HPC):
                pair, side = h // 2, h % 2
                poff = side * 64
                for qc in range(4):
                    c_ext = cxps.tile([65, 512], f32, tag="cext", name="c_ext")
                    for g in range(qc + 1):
                        sc = scps.tile([128, 2048], f32, tag="sc", name="sc_ps")
                        for j in range(4):
                            kt = 4 * g + j
                            nc.tensor.matmul(
                                sc[:, j * 512:(j + 1) * 512],
                                lhsT=k_sb[pair][poff:poff + 64,
                                                kt * 128:(kt + 1) * 128],
                                rhs=q_sb[pair][poff:poff + 64,
                                               qc * 512:(qc + 1) * 512],
                                start=True, stop=True)
                        e = esb.tile([128, 2048], f32, tag="e", name="e_sb")
                        nc.scalar.activation(e, sc, AF.Exp, scale=0.125)
                        if g == qc:
                            nc.vector.tensor_mul(e, e, mask_sb)
                        for j in range(4):
                            kt = 4 * g + j
                            vb = (pair * TK + kt) * V2W + side * 65
                            nc.tensor.matmul(
                                c_ext, lhsT=v2_sb[:, vb:vb + 65],
                                rhs=e[:, j * 512:(j + 1) * 512],
                                start=(g == 0 and j == 0),
                                stop=(g == qc and j == 3))
                    # engine APs need 32-aligned partition bases, so the
                    # denominator row (psum partition 64) is staged on
                    # partition 64 and moved to collector row by DMA.
                    row = h * 4 + qc
                    dst = stg.tile([65, 512], f32, tag="dstage", name="dstage")
                    nc.scalar.activation(dst[64:65, :], c_ext[64:65, :],
                                         AF.Copy)
                    nc.sync.dma_start(out=coll[row:row + 1, :],
                                      in_=dst[64:65, :])
                    nc.scalar.activation(
                        ctx_sb[pair][poff:poff + 64, qc * 512:(qc + 1) * 512],
                        c_ext[0:64, :], AF.Copy)

        # ---------------- phase 4: softmax normalization ----------------
        with tc.tile_pool(name="bcps", bufs=2, space="PSUM") as bcps, \
             tc.tile_pool(name="selp", bufs=1) as selp:
            sel_sb = selp.tile([32, 2048], f32, tag="sel", name="sel_sb")
            nc.sync.dma_start(out=sel_sb, in_=sel_d)
            nc.vector.reciprocal(collr, coll)
            for h in range(HPC):
                pair, side = h // 2, h % 2
                poff = side * 64
                for qc in range(4):
                    row = h * 4 + qc
                    bc = bcps.tile([64, 512], f32, tag="bc", name="bc_ps")
                    nc.tensor.matmul(
                        bc, lhsT=sel_sb[:, row * 64:(row + 1) * 64],
                        rhs=collr, start=True, stop=True)
                    cslice = ctx_sb[pair][poff:poff + 64,
                                          qc * 512:(qc + 1) * 512]
                    nc.vector.tensor_mul(cslice, cslice, bc)

        # ---------------- phase 5: out projection ----------------
        with tc.tile_pool(name="wop", bufs=1) as wop, \
             tc.tile_pool(name="yps", bufs=4, space="PSUM") as yps, \
             tc.tile_pool(name="ysbp", bufs=4) as ysbp:
            w_o_sb = [wop.tile([128, C], f32, tag=f"wo{i}", name=f"wo_sb{i}") for i in range(4)]
            for f in range(4):
                nc.sync.dma_start(out=w_o_sb[f],
                                  in_=w_o[f * 128:(f + 1) * 128, :])
            for tt in range(TK):
                for oc in range(2):
                    yp = yps.tile([128, 512], f32, tag="yp", name="y_ps")
                    for f in range(4):
                        nc.tensor.matmul(
                            yp, lhsT=ctx_sb[f][:, tt * 128:(tt + 1) * 128],
                            rhs=w_o_sb[f][:, oc * 512:(oc + 1) * 512],
                            start=(f == 0), stop=(f == 3))
                    ysb = ysbp.tile([128, 512], f32, tag="ysb", name="y_sb")
                    if (tt * 2 + oc) % 2 == 0:
                        nc.scalar.activation(ysb, yp, AF.Copy)
                    else:
                        nc.vector.tensor_copy(ysb, yp)
                    nc.sync.dma_start(
                        out=y_d[tt * 128:(tt + 1) * 128,
                                oc * 512:(oc + 1) * 512],
                        in_=ysb)

    nc.compile()
    return nc


def _host_inputs(x, w_qkv, b_qkv, w_out):
    """Build the 8 per-core input maps."""
    # causal mask for the diagonal ktile-group: mask[kp, j*512+qq] = (j*128+kp <= qq)
    kp = np.arange(128)[:, None]
    qq = np.arange(512)[None, :]
    blocks = [((j * 128 + kp) <= qq).astype(np.float32) for j in range(4)]
    mask = np.concatenate(blocks, axis=1)          # [128, 2048]
    sel = np.repeat(np.eye(32, dtype=np.float32), 64, axis=1)  # [32, 2048]

    xt = [np.ascontiguousarray(x[b].T) for b in range(B)]      # [C, T] each
    in_maps = []
    for core in range(NCORES):
        b, hg = core // 2, core % 2
        cs = slice(hg * FQ, (hg + 1) * FQ)
        w_slice = np.concatenate(
            [w_qkv[:, cs], w_qkv[:, C + hg * FQ: C + (hg + 1) * FQ],
             w_qkv[:, 2 * C + hg * FQ: 2 * C + (hg + 1) * FQ]], axis=1)
        b_slice = np.concatenate(
            [b_qkv[cs], b_qkv[C + hg * FQ: C + (hg + 1) * FQ],
             b_qkv[2 * C + hg * FQ: 2 * C + (hg + 1) * FQ]])
        in_maps.append({
            "x_t": xt[b],
            "w_s": np.ascontiguousarray(w_slice),
            "b_s": np.ascontiguousarray(b_slice),
            "w_o": np.ascontiguousarray(w_out[hg * FQ:(hg + 1) * FQ, :]),
            "mask": mask,
            "sel": sel,
        })
    return in_maps


def get_program():
    if "nc" not in _CACHE:
        _CACHE["nc"] = _build_program()
    return _CACHE["nc"]


def kernel(x, w_qkv, b_qkv, w_out, b_out):
    from concourse.bass_utils import run_bass_kernel_spmd

    x = np.asarray(x, dtype=np.float32)
    w_qkv = np.asarray(w_qkv, dtype=np.float32)
    b_qkv = np.asarray(b_qkv, dtype=np.float32)
    w_out = np.asarray(w_out, dtype=np.float32)
    b_out = np.asarray(b_out, dtype=np.float32)

    nc = get_program()
    in_maps = _host_inputs(x, w_qkv, b_qkv, w_out)
    res = run_bass_kernel_spmd(nc, in_maps, core_ids=list(range(NCORES)))

    out = np.empty((B, T, C), dtype=np.float32)
    for b in range(B):
        out[b] = res.results[2 * b]["y"] + res.results[2 * b + 1]["y"] + b_out
    return out


# revision 15
# speedup vs baseline: 2.5606x; 2.5606x over previous
"""Causal self-attention (B=4, T=2048, C=1024, H=16, D=64) on 8 trn2 cores.

Sharding: core c handles batch b = c//2 and head-group hg = c%2 (8 heads).
qkv projection is column-parallel, attention is head-parallel, out_proj is
row-parallel; the final 2-way partial-sum + bias happens on host.

Per-core device program (identical SPMD program, different data):
  phase 1: qkvT = (W_slice.T @ x.T) + bias   -> qT,kT [feat, tok], vT [feat, tok]
  phase 2: v2   = transpose(vT) interleaved with ones columns (softmax denom trick)
  phase 3: per (head, q-chunk): scoresT = kT.T@qT (causal-skipped), exp on ACT,
           causal mask on DVE, ctxT_ext = [v|1].T @ exp accumulated in PSUM
           (row 64 of ctxT_ext = softmax denominator, for free)
  phase 4: batched reciprocal of all denominators, broadcast via K=32 selector
           matmul, normalize ctx in place
  phase 5: y_partial = ctx_stacked.T @ W_out_slice -> DRAM
"""

import os
import sys

for _p in ("/opt/trn_rl_repo", "/root/.axon_site/_ro/trn_rl_repo"):
    if os.path.isdir(_p) and _p not in sys.path:
        sys.path.insert(0, _p)

import numpy as np

B, T, C = 4, 2048, 1024
H, D = 16, 64
NCORES = 8
HPC = 8          # heads per core
FQ = HPC * D     # 512 per-core q (=k=v) feature count
TK = T // 128    # 16 token tiles of 128
QC = T // 512    # 4 q-chunks of 512
V2W = 130        # v2 per-ktile width: 64 + 1 + 64 + 1

_CACHE = {}


def _build_program():
    import concourse.bacc as bacc
    import concourse.tile as tile
    import concourse.mybir as mybir
    from contextlib import ExitStack

    f32 = mybir.dt.float32
    f32r = mybir.dt.float32r
    AF = mybir.ActivationFunctionType

    nc = bacc.Bacc("TRN2", target_bir_lowering=False, debug=False)

    x_t = nc.dram_tensor("x_t", [C, T], f32r, kind="ExternalInput").ap()
    w_s = nc.dram_tensor("w_s", [C, 3 * FQ], f32r, kind="ExternalInput").ap()
    b_s = nc.dram_tensor("b_s", [3 * FQ], f32, kind="ExternalInput").ap()
    w_o = nc.dram_tensor("w_o", [FQ, C], f32r, kind="ExternalInput").ap()
    mask_d = nc.dram_tensor("mask", [128, 2048], f32, kind="ExternalInput").ap()
    sel_d = nc.dram_tensor("sel", [32, 2048], f32r, kind="ExternalInput").ap()
    idm_d = nc.dram_tensor("idm", [128, 128], f32r, kind="ExternalInput").ap()
    onec_d = nc.dram_tensor("onec", [128, 64], f32r, kind="ExternalInput").ap()
    y_d = nc.dram_tensor("y", [T, C], f32, kind="ExternalOutput").ap()

    with tile.TileContext(nc) as tc, ExitStack() as ctx:
        # ---- pools by lifetime ----
        # whole-kernel persistents
        pp = ctx.enter_context(tc.tile_pool(name="persist", bufs=1))
        ident = pp.tile([128, 128], f32r, tag="ident", name="ident")
        nc.sync.dma_start(out=ident, in_=idm_d)
        b_sb = pp.tile([128, 12], f32, tag="bias", name="b_sb")
        nc.sync.dma_start(out=b_sb, in_=b_s.rearrange("(f p) -> p f", p=128))
        coll = pp.tile([32, 512], f32, tag="coll", name="coll")
        collr = pp.tile([32, 512], f32r, tag="collr", name="collr")

        # q/k live phases 1-3; v2 lives 2-3; vT lives 1-2; ctx lives 3-5
        qk_pool = ctx.enter_context(tc.tile_pool(name="qk", bufs=1))
        q_sb = [qk_pool.tile([128, T], f32r, tag=f"q{i}", name=f"q_sb{i}") for i in range(4)]
        k_sb = [qk_pool.tile([128, T], f32r, tag=f"k{i}", name=f"k_sb{i}") for i in range(4)]
        v2_pool = ctx.enter_context(tc.tile_pool(name="v2p", bufs=1))
        v2_sb = v2_pool.tile([128, 4 * TK * V2W], f32r, tag="v2", name="v2_sb")
        # ones columns (softmax-denominator trick): strided DMA from host since
        # memset cannot produce f32r-rounded output
        v2v = v2_sb.rearrange("p (t w) -> p t w", w=V2W)
        onec3 = onec_d.rearrange("p (t o) -> p t o", o=1)
        nc.sync.dma_start(out=v2v[:, :, 64:65], in_=onec3)
        nc.sync.dma_start(out=v2v[:, :, 129:130], in_=onec3)

        # ---------------- phase 1: qkv projection ----------------
        with tc.tile_pool(name="xres", bufs=1) as xp, \
             tc.tile_pool(name="wstr", bufs=2) as wp, \
             tc.tile_pool(name="vTp", bufs=1) as vTp, \
             tc.tile_pool(name="ps1", bufs=2, space="PSUM") as ps1, \
             tc.tile_pool(name="pst", bufs=4, space="PSUM") as pst:
            x_sb = [xp.tile([128, T], f32r, tag=f"x{ks}", name=f"x_sb{ks}") for ks in range(8)]
            for ks in range(8):
                nc.sync.dma_start(out=x_sb[ks], in_=x_t[ks * 128:(ks + 1) * 128, :])
            vT_sb = [vTp.tile([128, T], f32r, tag=f"vT{i}", name=f"vT_sb{i}") for i in range(4)]

            for ft in range(12):
                wt = [wp.tile([128, 128], f32r, tag=f"w{ks}", name=f"wt{ks}") for ks in range(8)]
                for ks in range(8):
                    nc.sync.dma_start(
                        out=wt[ks],
                        in_=w_s[ks * 128:(ks + 1) * 128, ft * 128:(ft + 1) * 128])
                if ft < 4:
                    dest = q_sb[ft]
                elif ft < 8:
                    dest = k_sb[ft - 4]
                else:
                    dest = vT_sb[ft - 8]
                for tcn in range(4):
                    ps = ps1.tile([128, 512], f32, tag="qkvps", name="qkv_ps")
                    for ks in range(8):
                        nc.tensor.matmul(
                            ps, lhsT=wt[ks],
                            rhs=x_sb[ks][:, tcn * 512:(tcn + 1) * 512],
                            start=(ks == 0), stop=(ks == 7))
                    # evacuate with fused bias add: out = 1.0*psum + b
                    nc.scalar.activation(
                        dest[:, tcn * 512:(tcn + 1) * 512], ps,
                        AF.Identity, bias=b_sb[:, ft:ft + 1], scale=1.0)

            # ---------------- phase 2: build v2 (transposed v + ones cols) ----
            for pr in range(4):
                for kt in range(TK):
                    pt = pst.tile([128, 128], f32r, tag="tp", name="tr_ps")
                    nc.tensor.transpose(
                        pt, vT_sb[pr][:, kt * 128:(kt + 1) * 128],
                        ident)
                    base = (pr * TK + kt) * V2W
                    nc.vector.tensor_copy(v2_sb[:, base:base + 64], pt[:, 0:64])
                    nc.vector.tensor_copy(
                        v2_sb[:, base + 65:base + 129], pt[:, 64:128])

        # ---------------- phase 3: attention ----------------
        # ctx + mask allocated now that the phase-1 pools are closed
        ctx_pool = ctx.enter_context(tc.tile_pool(name="ctxp", bufs=1))
        ctx_sb = [ctx_pool.tile([128, T], f32r, tag=f"c{i}", name=f"ctx_sb{i}") for i in range(4)]
        mask_pool = ctx.enter_context(tc.tile_pool(name="maskp", bufs=1))
        mask_sb = mask_pool.tile([128, 2048], f32, tag="mask", name="mask_sb")
        nc.sync.dma_start(out=mask_sb, in_=mask_d)
        with tc.tile_pool(name="scps", bufs=1, space="PSUM") as scps, \
             tc.tile_pool(name="cxps", bufs=2, space="PSUM") as cxps, \
             tc.tile_pool(name="stg", bufs=4) as stg, \
             tc.tile_pool(name="esb", bufs=3) as esb:
            for h in range(HPC):
                pair, side = h // 2, h % 2
                poff = side * 64
                for qc in range(4):
                    c_ext = cxps.tile([65, 512], f32, tag="cext", name="c_ext")
                    for g in range(qc + 1):
                        sc = scps.tile([128, 2048], f32, tag="sc", name="sc_ps")
                        for j in range(4):
                            kt = 4 * g + j
                            nc.tensor.matmul(
                                sc[:, j * 512:(j + 1) * 512],
                                lhsT=k_sb[pair][poff:poff + 64,
                                                kt * 128:(kt + 1) * 128],
                                rhs=q_sb[pair][poff:poff + 64,
                                               qc * 512:(qc + 1) * 512],
                                start=True, stop=True)
                        e = esb.tile([128, 2048], f32r, tag="e", name="e_sb")
                        nc.scalar.activation(e, sc, AF.Exp, scale=0.125)
                        if g == qc:
                            nc.vector.tensor_mul(e, e, mask_sb)
                        for j in range(4):
                            kt = 4 * g + j
                            vb = (pair * TK + kt) * V2W + side * 65
                            nc.tensor.matmul(
                                c_ext, lhsT=v2_sb[:, vb:vb + 65],
                                rhs=e[:, j * 512:(j + 1) * 512],
                                start=(g == 0 and j == 0),
                                stop=(g == qc and j == 3))
                    # engine APs need 32-aligned partition bases, so the
                    # denominator row (psum partition 64) is staged on
                    # partition 64 and moved to collector row by DMA.
                    row = h * 4 + qc
                    dst = stg.tile([65, 512], f32, tag="dstage", name="dstage")
                    nc.scalar.activation(dst[64:65, :], c_ext[64:65, :],
                                         AF.Copy)
                    nc.sync.dma_start(out=coll[row:row + 1, :],
                                      in_=dst[64:65, :])
                    nc.scalar.activation(
                        ctx_sb[pair][poff:poff + 64, qc * 512:(qc + 1) * 512],
                        c_ext[0:64, :], AF.Copy)

        # ---------------- phase 4: softmax normalization ----------------
        with tc.tile_pool(name="bcps", bufs=2, space="PSUM") as bcps, \
             tc.tile_pool(name="selp", bufs=1) as selp:
            sel_sb = selp.tile([32, 2048], f32r, tag="sel", name="sel_sb")
            nc.sync.dma_start(out=sel_sb, in_=sel_d)
            # f32r is bit-identical to f32 for DVE; tag only affects the PE path
            with nc.allow_low_precision(reason="f32r == f32 storage"):
                nc.vector.reciprocal(collr, coll)
            for h in range(HPC):
                pair, side = h // 2, h % 2
                poff = side * 64
                for qc in range(4):
                    row = h * 4 + qc
                    bc = bcps.tile([64, 512], f32, tag="bc", name="bc_ps")
                    nc.tensor.matmul(
                        bc, lhsT=sel_sb[:, row * 64:(row + 1) * 64],
                        rhs=collr, start=True, stop=True)
                    cslice = ctx_sb[pair][poff:poff + 64,
                                          qc * 512:(qc + 1) * 512]
                    nc.vector.tensor_mul(cslice, cslice, bc)

        # ---------------- phase 5: out projection ----------------
        with tc.tile_pool(name="wop", bufs=1) as wop, \
             tc.tile_pool(name="yps", bufs=4, space="PSUM") as yps, \
             tc.tile_pool(name="ysbp", bufs=4) as ysbp:
            w_o_sb = [wop.tile([128, C], f32r, tag=f"wo{i}", name=f"wo_sb{i}") for i in range(4)]
            for f in range(4):
                nc.sync.dma_start(out=w_o_sb[f],
                                  in_=w_o[f * 128:(f + 1) * 128, :])
            for tt in range(TK):
                for oc in range(2):
                    yp = yps.tile([128, 512], f32, tag="yp", name="y_ps")
                    for f in range(4):
                        nc.tensor.matmul(
                            yp, lhsT=ctx_sb[f][:, tt * 128:(tt + 1) * 128],
                            rhs=w_o_sb[f][:, oc * 512:(oc + 1) * 512],
                            start=(f == 0), stop=(f == 3))
                    ysb = ysbp.tile([128, 512], f32, tag="ysb", name="y_sb")
                    if (tt * 2 + oc) % 2 == 0:
                        nc.scalar.activation(ysb, yp, AF.Copy)
                    else:
                        nc.vector.tensor_copy(ysb, yp)
                    nc.sync.dma_start(
                        out=y_d[tt * 128:(tt + 1) * 128,
                                oc * 512:(oc + 1) * 512],
                        in_=ysb)

    nc.compile()
    return nc


def _host_inputs(x, w_qkv, b_qkv, w_out):
    """Build the 8 per-core input maps."""
    # causal mask for the diagonal ktile-group: mask[kp, j*512+qq] = (j*128+kp <= qq)
    kp = np.arange(128)[:, None]
    qq = np.arange(512)[None, :]
    blocks = [((j * 128 + kp) <= qq).astype(np.float32) for j in range(4)]
    mask = np.concatenate(blocks, axis=1)          # [128, 2048]
    sel = np.repeat(np.eye(32, dtype=np.float32), 64, axis=1)  # [32, 2048]

    xt = [np.ascontiguousarray(x[b].T) for b in range(B)]      # [C, T] each
    in_maps = []
    for core in range(NCORES):
        b, hg = core // 2, core % 2
        cs = slice(hg * FQ, (hg + 1) * FQ)
        w_slice = np.concatenate(
            [w_qkv[:, cs], w_qkv[:, C + hg * FQ: C + (hg + 1) * FQ],
             w_qkv[:, 2 * C + hg * FQ: 2 * C + (hg + 1) * FQ]], axis=1)
        b_slice = np.concatenate(
            [b_qkv[cs], b_qkv[C + hg * FQ: C + (hg + 1) * FQ],
             b_qkv[2 * C + hg * FQ: 2 * C + (hg + 1) * FQ]])
        in_maps.append({
            "x_t": xt[b],
            "w_s": np.ascontiguousarray(w_slice),
            "b_s": np.ascontiguousarray(b_slice),
            "w_o": np.ascontiguousarray(w_out[hg * FQ:(hg + 1) * FQ, :]),
            "mask": mask,
            "sel": sel,
            "idm": np.eye(128, dtype=np.float32),
            "onec": np.ones((128, 64), dtype=np.float32),
        })
    return in_maps


def get_program():
    if "nc" not in _CACHE:
        _CACHE["nc"] = _build_program()
    return _CACHE["nc"]


def kernel(x, w_qkv, b_qkv, w_out, b_out):
    from concourse.bass_utils import run_bass_kernel_spmd

    x = np.asarray(x, dtype=np.float32)
    w_qkv = np.asarray(w_qkv, dtype=np.float32)
    b_qkv = np.asarray(b_qkv, dtype=np.float32)
    w_out = np.asarray(w_out, dtype=np.float32)
    b_out = np.asarray(b_out, dtype=np.float32)

    nc = get_program()
    in_maps = _host_inputs(x, w_qkv, b_qkv, w_out)
    res = run_bass_kernel_spmd(nc, in_maps, core_ids=list(range(NCORES)))

    out = np.empty((B, T, C), dtype=np.float32)
    for b in range(B):
        out[b] = res.results[2 * b]["y"] + res.results[2 * b + 1]["y"] + b_out
    return out


# revision 29
# speedup vs baseline: 3.5090x; 1.3704x over previous
"""Causal self-attention (B=4, T=2048, C=1024, H=16, D=64) on 8 trn2 cores.

Sharding: core c handles batch b = c//2 and head-group hg = c%2 (8 heads).
qkv projection is column-parallel, attention is head-parallel, out_proj is
row-parallel; the final 2-way partial-sum + bias happens on host.

Per-core device program (identical SPMD program, different data):
  phase 1: qkvT = (W_slice.T @ x.T) + bias   -> qT,kT [feat, tok], vT [feat, tok]
  phase 2: v2   = transpose(vT) interleaved with ones columns (softmax denom trick)
  phase 3: per (head, q-chunk): scoresT = kT.T@qT (causal-skipped), exp on ACT,
           causal mask on DVE, ctxT_ext = [v|1].T @ exp accumulated in PSUM
           (row 64 of ctxT_ext = softmax denominator, for free)
  phase 4: batched reciprocal of all denominators, broadcast via K=32 selector
           matmul, normalize ctx in place
  phase 5: y_partial = ctx_stacked.T @ W_out_slice -> DRAM
"""

import os
import sys

for _p in ("/opt/trn_rl_repo", "/root/.axon_site/_ro/trn_rl_repo"):
    if os.path.isdir(_p) and _p not in sys.path:
        sys.path.insert(0, _p)

import numpy as np

B, T, C = 4, 2048, 1024
H, D = 16, 64
NCORES = 8
HPC = 8          # heads per core
FQ = HPC * D     # 512 per-core q (=k=v) feature count
TK = T // 128    # 16 token tiles of 128
QC = T // 512    # 4 q-chunks of 512
V2W = 130        # v2 per-ktile width: 64 + 1 + 64 + 1

_CACHE = {}


def _build_program():
    import concourse.bacc as bacc
    import concourse.tile as tile
    import concourse.mybir as mybir
    from contextlib import ExitStack

    f32 = mybir.dt.float32
    f32r = mybir.dt.float32r
    AF = mybir.ActivationFunctionType

    nc = bacc.Bacc("TRN2", target_bir_lowering=False, debug=False)

    x_t = nc.dram_tensor("x_t", [C, T], f32r, kind="ExternalInput").ap()
    w_s = nc.dram_tensor("w_s", [C, 3 * FQ], f32r, kind="ExternalInput").ap()
    b_s = nc.dram_tensor("b_s", [3 * FQ], f32, kind="ExternalInput").ap()
    w_o = nc.dram_tensor("w_o", [FQ, C], f32r, kind="ExternalInput").ap()
    sel_d = nc.dram_tensor("sel", [32, 2048], f32r, kind="ExternalInput").ap()
    mask_d = nc.dram_tensor("mask", [128, 2048], f32, kind="ExternalInput").ap()
    idm_d = nc.dram_tensor("idm", [128, 128], f32r, kind="ExternalInput").ap()
    onec_d = nc.dram_tensor("onec", [128, 64], f32r, kind="ExternalInput").ap()
    y_d = nc.dram_tensor("y", [T, C], f32, kind="ExternalOutput").ap()

    with tile.TileContext(nc) as tc, ExitStack() as ctx:
        # ---- pools by lifetime ----
        # whole-kernel persistents
        pp = ctx.enter_context(tc.tile_pool(name="persist", bufs=1))
        ident = pp.tile([128, 128], f32r, tag="ident", name="ident")
        nc.sync.dma_start(out=ident, in_=idm_d)
        b_sb = pp.tile([128, 12], f32, tag="bias", name="b_sb")
        nc.sync.dma_start(out=b_sb, in_=b_s.rearrange("(f p) -> p f", p=128))
        # q/k live phases 1-3; v2 lives 2-3; vT lives 1-2; ctx lives 3-5
        qk_pool = ctx.enter_context(tc.tile_pool(name="qk", bufs=1))
        q_sb = [qk_pool.tile([128, T], f32r, tag=f"q{i}", name=f"q_sb{i}") for i in range(4)]
        k_sb = [qk_pool.tile([128, T], f32r, tag=f"k{i}", name=f"k_sb{i}") for i in range(4)]
        v2_pool = ctx.enter_context(tc.tile_pool(name="v2p", bufs=1))
        v2_sb = v2_pool.tile([128, 4 * TK * V2W], f32r, tag="v2", name="v2_sb")
        # ones columns (softmax-denominator trick): strided DMA from host since
        # memset cannot produce f32r-rounded output
        v2v = v2_sb.rearrange("p (t w) -> p t w", w=V2W)
        onec3 = onec_d.rearrange("p (t o) -> p t o", o=1)
        nc.sync.dma_start(out=v2v[:, :, 64:65], in_=onec3)
        nc.sync.dma_start(out=v2v[:, :, 129:130], in_=onec3)

        # ---------------- phase 1: qkv projection ----------------
        with tc.tile_pool(name="xres", bufs=1) as xp, \
             tc.tile_pool(name="wstr", bufs=2) as wp, \
             tc.tile_pool(name="vTp", bufs=1) as vTp, \
             tc.tile_pool(name="ps1", bufs=2, space="PSUM") as ps1, \
             tc.tile_pool(name="pst", bufs=4, space="PSUM") as pst:
            x_sb = [xp.tile([128, T], f32r, tag=f"x{ks}", name=f"x_sb{ks}") for ks in range(8)]
            # chunked tcn-outer so the first token-chunk's 8 k-slices arrive
            # quickly and the first psum accumulation can start early
            for tcn in range(4):
                for ks in range(8):
                    nc.sync.dma_start(
                        out=x_sb[ks][:, tcn * 512:(tcn + 1) * 512],
                        in_=x_t[ks * 128:(ks + 1) * 128, tcn * 512:(tcn + 1) * 512])
            vT_sb = [vTp.tile([128, T], f32r, tag=f"vT{i}", name=f"vT_sb{i}") for i in range(4)]

            for ft in (8, 9, 10, 11, 0, 1, 2, 3, 4, 5, 6, 7):
                # weights go on the gpsimd DMA queue so they don't queue
                # behind the bulk x transfer on the sync queue
                wt = [wp.tile([128, 128], f32r, tag=f"w{ks}", name=f"wt{ks}") for ks in range(8)]
                for ks in range(8):
                    nc.gpsimd.dma_start(
                        out=wt[ks],
                        in_=w_s[ks * 128:(ks + 1) * 128, ft * 128:(ft + 1) * 128])
                if ft < 4:
                    dest = q_sb[ft]
                elif ft < 8:
                    dest = k_sb[ft - 4]
                else:
                    dest = vT_sb[ft - 8]
                for tcn in range(4):
                    ps = ps1.tile([128, 512], f32, tag="qkvps", name="qkv_ps")
                    for ks in range(8):
                        nc.tensor.matmul(
                            ps, lhsT=wt[ks],
                            rhs=x_sb[ks][:, tcn * 512:(tcn + 1) * 512],
                            start=(ks == 0), stop=(ks == 7))
                    # evacuate with fused bias add: out = 1.0*psum + b
                    nc.scalar.activation(
                        dest[:, tcn * 512:(tcn + 1) * 512], ps,
                        AF.Identity, bias=b_sb[:, ft:ft + 1], scale=1.0)
                    if ft >= 8:
                        # v2 build interleaved: transpose the 4 ktiles of this
                        # freshly written v token-chunk
                        pr = ft - 8
                        for kt in range(4 * tcn, 4 * tcn + 4):
                            pt = pst.tile([128, 128], f32r, tag="tp", name="tr_ps")
                            nc.tensor.transpose(
                                pt, vT_sb[pr][:, kt * 128:(kt + 1) * 128],
                                ident)
                            base = (pr * TK + kt) * V2W
                            nc.vector.tensor_copy(v2_sb[:, base:base + 64], pt[:, 0:64])
                            nc.vector.tensor_copy(
                                v2_sb[:, base + 65:base + 129], pt[:, 64:128])

        # ---------------- phase 3: attention ----------------
        # ctx + mask allocated now that the phase-1 pools are closed
        ctx_pool = ctx.enter_context(tc.tile_pool(name="ctxp", bufs=1))
        ctx_sb = [ctx_pool.tile([128, T], f32r, tag=f"c{i}", name=f"ctx_sb{i}") for i in range(4)]
        mask_pool = ctx.enter_context(tc.tile_pool(name="maskp", bufs=1))
        mask_sb = mask_pool.tile([128, 2048], f32, tag="mask", name="mask_sb")
        nc.sync.dma_start(out=mask_sb, in_=mask_d)
        sel_pool = ctx.enter_context(tc.tile_pool(name="selp", bufs=1))
        sel_sb = sel_pool.tile([32, 2048], f32r, tag="sel", name="sel_sb")
        nc.gpsimd.dma_start(out=sel_sb, in_=sel_d)
        wo_pool = ctx.enter_context(tc.tile_pool(name="wop", bufs=1))
        w_o_sb = [wo_pool.tile([128, C], f32r, tag=f"wo{i}", name=f"wo_sb{i}") for i in range(4)]
        for f in range(4):
            nc.gpsimd.dma_start(out=w_o_sb[f], in_=w_o[f * 128:(f + 1) * 128, :])

        # per-qc denominator collector (partition base 0, 8 heads), rotated
        coll_pool = ctx.enter_context(tc.tile_pool(name="collp", bufs=2))

        # qc-outer: attention for all heads of one q-chunk, then that chunk's
        # softmax normalization and out-projection overlap the next chunk.
        with tc.tile_pool(name="scps", bufs=2, space="PSUM") as scps, \
             tc.tile_pool(name="cxps", bufs=2, space="PSUM") as cxps, \
             tc.tile_pool(name="bcps", bufs=1, space="PSUM") as bcps, \
             tc.tile_pool(name="yps", bufs=1, space="PSUM") as yps, \
             tc.tile_pool(name="stg", bufs=2) as stg, \
             tc.tile_pool(name="ysbp", bufs=4) as ysbp, \
             tc.tile_pool(name="esb", bufs=4) as esb:
            for qc in range(4):
                collq = coll_pool.tile([8, 512], f32, tag="cq", name="collq")
                collrq = coll_pool.tile([8, 512], f32r, tag="cr", name="collrq")
                for h in range(HPC):
                    pair, side = h // 2, h % 2
                    poff = side * 64
                    c_ext = cxps.tile([65, 512], f32, tag="cext", name="c_ext")
                    ngr = 2 * (qc + 1)      # groups of 2 ktiles
                    for g in range(ngr):
                        sc = scps.tile([128, 1024], f32, tag="sc", name="sc_ps")
                        for j in range(2):
                            kt = 2 * g + j
                            nc.tensor.matmul(
                                sc[:, j * 512:(j + 1) * 512],
                                lhsT=k_sb[pair][poff:poff + 64,
                                                kt * 128:(kt + 1) * 128],
                                rhs=q_sb[pair][poff:poff + 64,
                                               qc * 512:(qc + 1) * 512],
                                start=True, stop=True)
                        e = esb.tile([128, 1024], f32r, tag="e", name="e_sb")
                        nc.scalar.activation(e, sc, AF.Exp, scale=0.125)
                        if g >= ngr - 2:
                            # diagonal ktile pair: apply causal mask
                            m = g - (ngr - 2)
                            nc.vector.tensor_mul(
                                e, e, mask_sb[:, m * 1024:m * 1024 + 1024])
                        for j in range(2):
                            kt = 2 * g + j
                            vb = (pair * TK + kt) * V2W + side * 65
                            nc.tensor.matmul(
                                c_ext, lhsT=v2_sb[:, vb:vb + 65],
                                rhs=e[:, j * 512:(j + 1) * 512],
                                start=(g == 0 and j == 0),
                                stop=(g == ngr - 1 and j == 1))
                    # engine APs need 32-aligned partition bases, so the
                    # denominator row (psum partition 64) is staged on
                    # partition 64 and moved to collector row by DMA.
                    dst = stg.tile([65, 512], f32, tag="dstage", name="dstage")
                    nc.vector.tensor_copy(dst[64:65, :], c_ext[64:65, :])
                    nc.sync.dma_start(out=collq[h:h + 1, :],
                                      in_=dst[64:65, :])
                    nc.vector.tensor_copy(
                        ctx_sb[pair][poff:poff + 64, qc * 512:(qc + 1) * 512],
                        c_ext[0:64, :])

                # normalization for this q-chunk
                with nc.allow_low_precision(reason="f32r == f32 storage"):
                    nc.vector.reciprocal(collrq, collq)
                for h in range(HPC):
                    pair, side = h // 2, h % 2
                    poff = side * 64
                    bc = bcps.tile([64, 512], f32, tag="bc", name="bc_ps")
                    nc.tensor.matmul(
                        bc, lhsT=sel_sb[0:8, h * 64:(h + 1) * 64],
                        rhs=collrq, start=True, stop=True)
                    cslice = ctx_sb[pair][poff:poff + 64,
                                          qc * 512:(qc + 1) * 512]
                    nc.vector.tensor_mul(cslice, cslice, bc)

                # out-projection for this q-chunk's 4 token tiles
                for tt in range(4 * qc, 4 * qc + 4):
                    for oc in range(2):
                        yp = yps.tile([128, 512], f32, tag="yp", name="y_ps")
                        for f in range(4):
                            nc.tensor.matmul(
                                yp, lhsT=ctx_sb[f][:, tt * 128:(tt + 1) * 128],
                                rhs=w_o_sb[f][:, oc * 512:(oc + 1) * 512],
                                start=(f == 0), stop=(f == 3))
                        ysb = ysbp.tile([128, 512], f32, tag="ysb", name="y_sb")
                        if oc == 0:
                            nc.scalar.activation(ysb, yp, AF.Copy)
                        else:
                            nc.vector.tensor_copy(ysb, yp)
                        nc.sync.dma_start(
                            out=y_d[tt * 128:(tt + 1) * 128,
                                    oc * 512:(oc + 1) * 512],
                            in_=ysb)

    nc.compile()
    return nc


def _host_inputs(x, w_qkv, b_qkv, w_out):
    """Build the 8 per-core input maps."""
    sel = np.repeat(np.eye(32, dtype=np.float32), 64, axis=1)  # [32, 2048]
    kp = np.arange(128)[:, None]
    qq = np.arange(512)[None, :]
    blocks = [((j * 128 + kp) <= qq).astype(np.float32) for j in range(4)]
    mask = np.concatenate(blocks, axis=1)          # [128, 2048]

    xt = [np.ascontiguousarray(x[b].T) for b in range(B)]      # [C, T] each
    in_maps = []
    for core in range(NCORES):
        b, hg = core // 2, core % 2
        cs = slice(hg * FQ, (hg + 1) * FQ)
        w_slice = np.concatenate(
            [w_qkv[:, cs], w_qkv[:, C + hg * FQ: C + (hg + 1) * FQ],
             w_qkv[:, 2 * C + hg * FQ: 2 * C + (hg + 1) * FQ]], axis=1)
        b_slice = np.concatenate(
            [b_qkv[cs], b_qkv[C + hg * FQ: C + (hg + 1) * FQ],
             b_qkv[2 * C + hg * FQ: 2 * C + (hg + 1) * FQ]])
        in_maps.append({
            "x_t": xt[b],
            "w_s": np.ascontiguousarray(w_slice),
            "b_s": np.ascontiguousarray(b_slice),
            "w_o": np.ascontiguousarray(w_out[hg * FQ:(hg + 1) * FQ, :]),
            "sel": sel,
            "mask": mask,
            "idm": np.eye(128, dtype=np.float32),
            "onec": np.ones((128, 64), dtype=np.float32),
        })
    return in_maps


def get_program():
    if "nc" not in _CACHE:
        _CACHE["nc"] = _build_program()
    return _CACHE["nc"]


def kernel(x, w_qkv, b_qkv, w_out, b_out):
    from concourse.bass_utils import run_bass_kernel_spmd

    x = np.asarray(x, dtype=np.float32)
    w_qkv = np.asarray(w_qkv, dtype=np.float32)
    b_qkv = np.asarray(b_qkv, dtype=np.float32)
    w_out = np.asarray(w_out, dtype=np.float32)
    b_out = np.asarray(b_out, dtype=np.float32)

    nc = get_program()
    in_maps = _host_inputs(x, w_qkv, b_qkv, w_out)
    res = run_bass_kernel_spmd(nc, in_maps, core_ids=list(range(NCORES)))

    out = np.empty((B, T, C), dtype=np.float32)
    for b in range(B):
        out[b] = res.results[2 * b]["y"] + res.results[2 * b + 1]["y"] + b_out
    return out


# revision 30
# speedup vs baseline: 3.7748x; 1.0758x over previous
"""Causal self-attention (B=4, T=2048, C=1024, H=16, D=64) on 8 trn2 cores.

Sharding: core c handles batch b = c//2 and head-group hg = c%2 (8 heads).
qkv projection is column-parallel, attention is head-parallel, out_proj is
row-parallel; the final 2-way partial-sum + bias happens on host.

Per-core device program (identical SPMD program, different data):
  phase 1: qkvT = (W_slice.T @ x.T) + bias   -> qT,kT [feat, tok], vT [feat, tok]
  phase 2: v2   = transpose(vT) interleaved with ones columns (softmax denom trick)
  phase 3: per (head, q-chunk): scoresT = kT.T@qT (causal-skipped), exp on ACT,
           causal mask on DVE, ctxT_ext = [v|1].T @ exp accumulated in PSUM
           (row 64 of ctxT_ext = softmax denominator, for free)
  phase 4: batched reciprocal of all denominators, broadcast via K=32 selector
           matmul, normalize ctx in place
  phase 5: y_partial = ctx_stacked.T @ W_out_slice -> DRAM
"""

import os
import sys

for _p in ("/opt/trn_rl_repo", "/root/.axon_site/_ro/trn_rl_repo"):
    if os.path.isdir(_p) and _p not in sys.path:
        sys.path.insert(0, _p)

import numpy as np

B, T, C = 4, 2048, 1024
H, D = 16, 64
NCORES = 8
HPC = 8          # heads per core
FQ = HPC * D     # 512 per-core q (=k=v) feature count
TK = T // 128    # 16 token tiles of 128
QC = T // 512    # 4 q-chunks of 512
V2W = 130        # v2 per-ktile width: 64 + 1 + 64 + 1

_CACHE = {}


def _build_program():
    import concourse.bacc as bacc
    import concourse.tile as tile
    import concourse.mybir as mybir
    from contextlib import ExitStack

    f32 = mybir.dt.float32
    f32r = mybir.dt.float32r
    AF = mybir.ActivationFunctionType

    nc = bacc.Bacc("TRN2", target_bir_lowering=False, debug=False)

    x_t = nc.dram_tensor("x_t", [C, T], f32r, kind="ExternalInput").ap()
    w_s = nc.dram_tensor("w_s", [C, 3 * FQ], f32r, kind="ExternalInput").ap()
    b_s = nc.dram_tensor("b_s", [3 * FQ], f32, kind="ExternalInput").ap()
    w_o = nc.dram_tensor("w_o", [FQ, C], f32r, kind="ExternalInput").ap()
    sel_d = nc.dram_tensor("sel", [32, 2048], f32r, kind="ExternalInput").ap()
    tri_d = nc.dram_tensor("tri", [128, 128], f32, kind="ExternalInput").ap()
    idm_d = nc.dram_tensor("idm", [128, 128], f32r, kind="ExternalInput").ap()
    onec_d = nc.dram_tensor("onec", [128, 64], f32r, kind="ExternalInput").ap()
    y_d = nc.dram_tensor("y", [T, C], f32, kind="ExternalOutput").ap()

    with tile.TileContext(nc) as tc, ExitStack() as ctx:
        # ---- pools by lifetime ----
        # whole-kernel persistents
        pp = ctx.enter_context(tc.tile_pool(name="persist", bufs=1))
        ident = pp.tile([128, 128], f32r, tag="ident", name="ident")
        nc.sync.dma_start(out=ident, in_=idm_d)
        b_sb = pp.tile([128, 12], f32, tag="bias", name="b_sb")
        nc.sync.dma_start(out=b_sb, in_=b_s.rearrange("(f p) -> p f", p=128))
        # q/k live phases 1-3; v2 lives 2-3; vT lives 1-2; ctx lives 3-5
        qk_pool = ctx.enter_context(tc.tile_pool(name="qk", bufs=1))
        q_sb = [qk_pool.tile([128, T], f32r, tag=f"q{i}", name=f"q_sb{i}") for i in range(4)]
        k_sb = [qk_pool.tile([128, T], f32r, tag=f"k{i}", name=f"k_sb{i}") for i in range(4)]
        v2_pool = ctx.enter_context(tc.tile_pool(name="v2p", bufs=1))
        v2_sb = v2_pool.tile([128, 4 * TK * V2W], f32r, tag="v2", name="v2_sb")
        # ones columns (softmax-denominator trick): strided DMA from host since
        # memset cannot produce f32r-rounded output
        v2v = v2_sb.rearrange("p (t w) -> p t w", w=V2W)
        onec3 = onec_d.rearrange("p (t o) -> p t o", o=1)
        nc.sync.dma_start(out=v2v[:, :, 64:65], in_=onec3)
        nc.sync.dma_start(out=v2v[:, :, 129:130], in_=onec3)

        # ---------------- phase 1: qkv projection ----------------
        with tc.tile_pool(name="xres", bufs=1) as xp, \
             tc.tile_pool(name="wstr", bufs=2) as wp, \
             tc.tile_pool(name="vTp", bufs=1) as vTp, \
             tc.tile_pool(name="ps1", bufs=2, space="PSUM") as ps1, \
             tc.tile_pool(name="pst", bufs=4, space="PSUM") as pst:
            x_sb = [xp.tile([128, T], f32r, tag=f"x{ks}", name=f"x_sb{ks}") for ks in range(8)]
            # chunked tcn-outer so the first token-chunk's 8 k-slices arrive
            # quickly and the first psum accumulation can start early
            for tcn in range(4):
                for ks in range(8):
                    nc.sync.dma_start(
                        out=x_sb[ks][:, tcn * 512:(tcn + 1) * 512],
                        in_=x_t[ks * 128:(ks + 1) * 128, tcn * 512:(tcn + 1) * 512])
            vT_sb = [vTp.tile([128, T], f32r, tag=f"vT{i}", name=f"vT_sb{i}") for i in range(4)]

            for ft in (8, 9, 10, 11, 0, 1, 2, 3, 4, 5, 6, 7):
                # weights go on the gpsimd DMA queue so they don't queue
                # behind the bulk x transfer on the sync queue
                wt = [wp.tile([128, 128], f32r, tag=f"w{ks}", name=f"wt{ks}") for ks in range(8)]
                for ks in range(8):
                    nc.gpsimd.dma_start(
                        out=wt[ks],
                        in_=w_s[ks * 128:(ks + 1) * 128, ft * 128:(ft + 1) * 128])
                if ft < 4:
                    dest = q_sb[ft]
                elif ft < 8:
                    dest = k_sb[ft - 4]
                else:
                    dest = vT_sb[ft - 8]
                for tcn in range(4):
                    ps = ps1.tile([128, 512], f32, tag="qkvps", name="qkv_ps")
                    for ks in range(8):
                        nc.tensor.matmul(
                            ps, lhsT=wt[ks],
                            rhs=x_sb[ks][:, tcn * 512:(tcn + 1) * 512],
                            start=(ks == 0), stop=(ks == 7))
                    # evacuate with fused bias add: out = 1.0*psum + b
                    nc.scalar.activation(
                        dest[:, tcn * 512:(tcn + 1) * 512], ps,
                        AF.Identity, bias=b_sb[:, ft:ft + 1], scale=1.0)
                    if ft >= 8:
                        # v2 build interleaved: transpose the 4 ktiles of this
                        # freshly written v token-chunk
                        pr = ft - 8
                        for kt in range(4 * tcn, 4 * tcn + 4):
                            pt = pst.tile([128, 128], f32r, tag="tp", name="tr_ps")
                            nc.tensor.transpose(
                                pt, vT_sb[pr][:, kt * 128:(kt + 1) * 128],
                                ident)
                            base = (pr * TK + kt) * V2W
                            nc.vector.tensor_copy(v2_sb[:, base:base + 64], pt[:, 0:64])
                            nc.vector.tensor_copy(
                                v2_sb[:, base + 65:base + 129], pt[:, 64:128])

        # ---------------- phase 3: attention ----------------
        # ctx + mask allocated now that the phase-1 pools are closed
        ctx_pool = ctx.enter_context(tc.tile_pool(name="ctxp", bufs=1))
        ctx_sb = [ctx_pool.tile([128, T], f32r, tag=f"c{i}", name=f"ctx_sb{i}") for i in range(4)]
        mask_pool = ctx.enter_context(tc.tile_pool(name="maskp", bufs=1))
        tri_sb = mask_pool.tile([128, 128], f32, tag="tri", name="tri_sb")
        nc.sync.dma_start(out=tri_sb, in_=tri_d)
        sel_pool = ctx.enter_context(tc.tile_pool(name="selp", bufs=1))
        sel_sb = sel_pool.tile([32, 2048], f32r, tag="sel", name="sel_sb")
        nc.gpsimd.dma_start(out=sel_sb, in_=sel_d)
        wo_pool = ctx.enter_context(tc.tile_pool(name="wop", bufs=1))
        w_o_sb = [wo_pool.tile([128, C], f32r, tag=f"wo{i}", name=f"wo_sb{i}") for i in range(4)]
        for f in range(4):
            nc.gpsimd.dma_start(out=w_o_sb[f], in_=w_o[f * 128:(f + 1) * 128, :])

        # per-qc denominator collector (partition base 0, 8 heads), rotated
        coll_pool = ctx.enter_context(tc.tile_pool(name="collp", bufs=2))

        # qc-outer: attention for all heads of one q-chunk, then that chunk's
        # softmax normalization and out-projection overlap the next chunk.
        with tc.tile_pool(name="scps", bufs=2, space="PSUM") as scps, \
             tc.tile_pool(name="cxps", bufs=2, space="PSUM") as cxps, \
             tc.tile_pool(name="bcps", bufs=1, space="PSUM") as bcps, \
             tc.tile_pool(name="yps", bufs=1, space="PSUM") as yps, \
             tc.tile_pool(name="stg", bufs=2) as stg, \
             tc.tile_pool(name="ysbp", bufs=4) as ysbp, \
             tc.tile_pool(name="esb", bufs=4) as esb:
            for qc in range(4):
                collq = coll_pool.tile([8, 512], f32, tag="cq", name="collq")
                collrq = coll_pool.tile([8, 512], f32r, tag="cr", name="collrq")
                ngr = 2 * (qc + 1)      # groups of 2 ktiles
                for pair in range(4):
                    # the two heads of a pair live on partition halves 0-63 /
                    # 64-127, so their K=64 scores matmuls land in different
                    # PE row-groups and can run concurrently on hardware.
                    c_ext = [cxps.tile([65, 512], f32, tag="cext",
                                       name="c_ext") for _ in range(2)]
                    for g in range(ngr):
                        diag = g >= ngr - 2
                        m = g - (ngr - 2)
                        scp = [scps.tile([128, 1024], f32, tag="sc",
                                         name="sc_ps") for _ in range(2)]
                        for j in range(2):
                            kt = 2 * g + j
                            roff = (2 * m + j) * 128 if diag else 0
                            for side in range(2):
                                poff = side * 64
                                nc.tensor.matmul(
                                    scp[side][:, j * 512 + roff:(j + 1) * 512],
                                    lhsT=k_sb[pair][poff:poff + 64,
                                                    kt * 128:(kt + 1) * 128],
                                    rhs=q_sb[pair][poff:poff + 64,
                                                   qc * 512 + roff:
                                                   (qc + 1) * 512],
                                    start=True, stop=True)
                        ee = []
                        for side in range(2):
                            e = esb.tile([128, 1024], f32r, tag="e",
                                         name="e_sb")
                            if diag and m == 1:
                                # only ktiles r=2,3 are valid here; exp just
                                # the live column ranges
                                nc.scalar.activation(
                                    e[:, 256:512], scp[side][:, 256:512],
                                    AF.Exp, scale=0.125)
                                nc.scalar.activation(
                                    e[:, 896:1024], scp[side][:, 896:1024],
                                    AF.Exp, scale=0.125)
                            else:
                                nc.scalar.activation(e, scp[side], AF.Exp,
                                                     scale=0.125)
                            if diag:
                                # in-tile causal boundary: 128-wide triangle
                                # per diagonal ktile
                                for j in range(2):
                                    r = 2 * m + j
                                    c0 = j * 512 + r * 128
                                    nc.vector.tensor_mul(
                                        e[:, c0:c0 + 128],
                                        e[:, c0:c0 + 128], tri_sb)
                            ee.append(e)
                        for j in range(2):
                            kt = 2 * g + j
                            r = 2 * m + j
                            roff = r * 128 if diag else 0
                            for side in range(2):
                                vb = (pair * TK + kt) * V2W + side * 65
                                nc.tensor.matmul(
                                    c_ext[side][:, roff:512],
                                    lhsT=v2_sb[:, vb:vb + 65],
                                    rhs=ee[side][:, j * 512 + roff:
                                                 (j + 1) * 512],
                                    start=(g == 0 and j == 0),
                                    stop=(g == ngr - 1 and j == 1))
                    for side in range(2):
                        h = 2 * pair + side
                        poff = side * 64
                        # engine APs need 32-aligned partition bases, so the
                        # denominator row (psum partition 64) is staged on
                        # partition 64 and moved to collector row by DMA.
                        dst = stg.tile([65, 512], f32, tag="dstage",
                                       name="dstage")
                        nc.vector.tensor_copy(dst[64:65, :],
                                              c_ext[side][64:65, :])
                        nc.sync.dma_start(out=collq[h:h + 1, :],
                                          in_=dst[64:65, :])
                        nc.vector.tensor_copy(
                            ctx_sb[pair][poff:poff + 64,
                                         qc * 512:(qc + 1) * 512],
                            c_ext[side][0:64, :])

                # normalization for this q-chunk
                with nc.allow_low_precision(reason="f32r == f32 storage"):
                    nc.vector.reciprocal(collrq, collq)
                for h in range(HPC):
                    pair, side = h // 2, h % 2
                    poff = side * 64
                    bc = bcps.tile([64, 512], f32, tag="bc", name="bc_ps")
                    nc.tensor.matmul(
                        bc, lhsT=sel_sb[0:8, h * 64:(h + 1) * 64],
                        rhs=collrq, start=True, stop=True)
                    cslice = ctx_sb[pair][poff:poff + 64,
                                          qc * 512:(qc + 1) * 512]
                    nc.vector.tensor_mul(cslice, cslice, bc)

                # out-projection for this q-chunk's 4 token tiles
                for tt in range(4 * qc, 4 * qc + 4):
                    for oc in range(2):
                        yp = yps.tile([128, 512], f32, tag="yp", name="y_ps")
                        for f in range(4):
                            nc.tensor.matmul(
                                yp, lhsT=ctx_sb[f][:, tt * 128:(tt + 1) * 128],
                                rhs=w_o_sb[f][:, oc * 512:(oc + 1) * 512],
                                start=(f == 0), stop=(f == 3))
                        ysb = ysbp.tile([128, 512], f32, tag="ysb", name="y_sb")
                        if oc == 0:
                            nc.scalar.activation(ysb, yp, AF.Copy)
                        else:
                            nc.vector.tensor_copy(ysb, yp)
                        nc.sync.dma_start(
                            out=y_d[tt * 128:(tt + 1) * 128,
                                    oc * 512:(oc + 1) * 512],
                            in_=ysb)

    nc.compile()
    return nc


def _host_inputs(x, w_qkv, b_qkv, w_out):
    """Build the 8 per-core input maps."""
    sel = np.repeat(np.eye(32, dtype=np.float32), 64, axis=1)  # [32, 2048]
    tri = (np.arange(128)[:, None] <= np.arange(128)[None, :]).astype(np.float32)

    xt = [np.ascontiguousarray(x[b].T) for b in range(B)]      # [C, T] each
    in_maps = []
    for core in range(NCORES):
        b, hg = core // 2, core % 2
        cs = slice(hg * FQ, (hg + 1) * FQ)
        w_slice = np.concatenate(
            [w_qkv[:, cs], w_qkv[:, C + hg * FQ: C + (hg + 1) * FQ],
             w_qkv[:, 2 * C + hg * FQ: 2 * C + (hg + 1) * FQ]], axis=1)
        b_slice = np.concatenate(
            [b_qkv[cs], b_qkv[C + hg * FQ: C + (hg + 1) * FQ],
             b_qkv[2 * C + hg * FQ: 2 * C + (hg + 1) * FQ]])
        in_maps.append({
            "x_t": xt[b],
            "w_s": np.ascontiguousarray(w_slice),
            "b_s": np.ascontiguousarray(b_slice),
            "w_o": np.ascontiguousarray(w_out[hg * FQ:(hg + 1) * FQ, :]),
            "sel": sel,
            "tri": tri,
            "idm": np.eye(128, dtype=np.float32),
            "onec": np.ones((128, 64), dtype=np.float32),
        })
    return in_maps


def get_program():
    if "nc" not in _CACHE:
        _CACHE["nc"] = _build_program()
    return _CACHE["nc"]


def kernel(x, w_qkv, b_qkv, w_out, b_out):
    from concourse.bass_utils import run_bass_kernel_spmd

    x = np.asarray(x, dtype=np.float32)
    w_qkv = np.asarray(w_qkv, dtype=np.float32)
    b_qkv = np.asarray(b_qkv, dtype=np.float32)
    w_out = np.asarray(w_out, dtype=np.float32)
    b_out = np.asarray(b_out, dtype=np.float32)

    nc = get_program()
    in_maps = _host_inputs(x, w_qkv, b_qkv, w_out)
    res = run_bass_kernel_spmd(nc, in_maps, core_ids=list(range(NCORES)))

    out = np.empty((B, T, C), dtype=np.float32)
    for b in range(B):
        out[b] = res.results[2 * b]["y"] + res.results[2 * b + 1]["y"] + b_out
    return out


# revision 37
# speedup vs baseline: 4.0597x; 1.0755x over previous
"""Causal self-attention (B=4, T=2048, C=1024, H=16, D=64) on 8 trn2 cores.

Sharding: core c handles batch b = c//2 and head-group hg = c%2 (8 heads).
qkv projection is column-parallel, attention is head-parallel, out_proj is
row-parallel; the final 2-way partial-sum + bias happens on host.

Per-core device program, pipelined over head PAIRS so the qkv projection of
pair p+1 overlaps the attention of pair p:
  per pair p (heads 2p, 2p+1, living on partition halves 0-63 / 64-127):
    - qkvT = (W_slice.T @ x.T) + bias -> qT,kT [feat, tok], vT [feat, tok]
    - v2 = PE-transpose(vT) with interleaved ones columns (the ones column
      makes the attn@v matmul also emit the softmax denominator row)
    - per q-chunk: scoresT = kT.T@qT (causal-skipped + sliced), exp on ACT,
      128-wide triangle mask on DVE, ctxT_ext = [v|1].T @ exp in PSUM;
      denominators collected by DMA, batched reciprocal, broadcast across
      partitions by a partition-step-0 DMA, normalize ctx in place
  tail: y_partial = ctx_stacked.T @ W_out_slice -> DRAM
"""

import os
import sys

for _p in ("/opt/trn_rl_repo", "/root/.axon_site/_ro/trn_rl_repo"):
    if os.path.isdir(_p) and _p not in sys.path:
        sys.path.insert(0, _p)

import numpy as np

B, T, C = 4, 2048, 1024
H, D = 16, 64
NCORES = 8
HPC = 8          # heads per core
FQ = HPC * D     # 512 per-core q (=k=v) feature count
TK = T // 128    # 16 token tiles of 128
V2W = 130        # v2 per-ktile width: 64 + 1 + 64 + 1

_CACHE = {}


def _build_program():
    import concourse.bacc as bacc
    import concourse.tile as tile
    import concourse.mybir as mybir
    from contextlib import ExitStack

    f32 = mybir.dt.float32
    f32r = mybir.dt.float32r
    AF = mybir.ActivationFunctionType

    nc = bacc.Bacc("TRN2", target_bir_lowering=False, debug=False)

    x_t = nc.dram_tensor("x_t", [C, T], f32r, kind="ExternalInput").ap()
    w_s = nc.dram_tensor("w_s", [C, 3 * FQ], f32r, kind="ExternalInput").ap()
    b_s = nc.dram_tensor("b_s", [3 * FQ], f32, kind="ExternalInput").ap()
    w_o = nc.dram_tensor("w_o", [FQ, C], f32r, kind="ExternalInput").ap()
    tri_d = nc.dram_tensor("tri", [128, 128], f32, kind="ExternalInput").ap()
    idm_d = nc.dram_tensor("idm", [128, 128], f32r, kind="ExternalInput").ap()
    onec_d = nc.dram_tensor("onec", [128, 64], f32r, kind="ExternalInput").ap()
    y_d = nc.dram_tensor("y", [T, C], f32, kind="ExternalOutput").ap()

    with tile.TileContext(nc) as tc, ExitStack() as ctx:
        # ---- whole-kernel persistents ----
        pp = ctx.enter_context(tc.tile_pool(name="persist", bufs=1))
        ident = pp.tile([128, 128], f32r, tag="ident", name="ident")
        nc.sync.dma_start(out=ident, in_=idm_d)
        b_sb = pp.tile([128, 12], f32, tag="bias", name="b_sb")
        nc.sync.dma_start(out=b_sb, in_=b_s.rearrange("(f p) -> p f", p=128))
        tri_sb = pp.tile([128, 128], f32, tag="tri", name="tri_sb")
        nc.sync.dma_start(out=tri_sb, in_=tri_d)

        ctx_pool = ctx.enter_context(tc.tile_pool(name="ctxp", bufs=1))
        ctx_sb = [ctx_pool.tile([128, T], f32r, tag=f"c{i}", name=f"ctx_sb{i}")
                  for i in range(4)]

        # psum pools that span the whole pair pipeline (8 banks total:
        # qkv 1 + transpose 1 + scores 4 + ctx 2)
        ps1 = ctx.enter_context(tc.tile_pool(name="ps1", bufs=1, space="PSUM"))
        pst = ctx.enter_context(tc.tile_pool(name="pst", bufs=1, space="PSUM"))

        with tc.tile_pool(name="xres", bufs=1) as xp, \
             tc.tile_pool(name="wstr", bufs=2) as wp, \
             tc.tile_pool(name="qkq", bufs=2) as qkq, \
             tc.tile_pool(name="vTq", bufs=2) as vTq, \
             tc.tile_pool(name="v2q", bufs=2) as v2q, \
             tc.tile_pool(name="collp", bufs=2) as coll_pool, \
             tc.tile_pool(name="bcq", bufs=2) as bcq, \
             tc.tile_pool(name="dscr", bufs=4, space="DRAM") as dscr, \
             tc.tile_pool(name="stg", bufs=2) as stg, \
             tc.tile_pool(name="esb", bufs=3) as esb, \
             tc.tile_pool(name="scps", bufs=2, space="PSUM") as scps, \
             tc.tile_pool(name="cxps", bufs=2, space="PSUM") as cxps:

            x_sb = [xp.tile([128, T], f32r, tag=f"x{ks}", name=f"x_sb{ks}")
                    for ks in range(8)]
            # chunked tcn-outer so the first token-chunk's 8 k-slices arrive
            # quickly and the first psum accumulation can start early
            for tcn in range(4):
                for ks in range(8):
                    nc.sync.dma_start(
                        out=x_sb[ks][:, tcn * 512:(tcn + 1) * 512],
                        in_=x_t[ks * 128:(ks + 1) * 128,
                                tcn * 512:(tcn + 1) * 512])

            for pair in range(4):
                # ---- qkv projection for this pair (v first, then q, k) ----
                qp = qkq.tile([128, T], f32r, tag="qp", name="q_p")
                kp = qkq.tile([128, T], f32r, tag="kp", name="k_p")
                vT = vTq.tile([128, T], f32r, tag="vT", name="vT_p")
                v2 = v2q.tile([128, TK * V2W], f32r, tag="v2", name="v2_p")
                v2v = v2.rearrange("p (t w) -> p t w", w=V2W)
                onec3 = onec_d[:, 0:16].rearrange("p (t o) -> p t o", o=1)
                nc.sync.dma_start(out=v2v[:, :, 64:65], in_=onec3)
                nc.sync.dma_start(out=v2v[:, :, 129:130], in_=onec3)

                for ft, dest in ((8 + pair, vT), (pair, qp), (4 + pair, kp)):
                    wt = [wp.tile([128, 128], f32r, tag=f"w{ks}",
                                  name=f"wt{ks}") for ks in range(8)]
                    for ks in range(8):
                        nc.gpsimd.dma_start(
                            out=wt[ks],
                            in_=w_s[ks * 128:(ks + 1) * 128,
                                    ft * 128:(ft + 1) * 128])
                    for tcn in range(4):
                        ps = ps1.tile([128, 512], f32, tag="qkvps",
                                      name="qkv_ps")
                        for ks in range(8):
                            nc.tensor.matmul(
                                ps, lhsT=wt[ks],
                                rhs=x_sb[ks][:, tcn * 512:(tcn + 1) * 512],
                                start=(ks == 0), stop=(ks == 7))
                        # evacuate with fused bias add: out = psum + b
                        nc.scalar.activation(
                            dest[:, tcn * 512:(tcn + 1) * 512], ps,
                            AF.Identity, bias=b_sb[:, ft:ft + 1], scale=1.0)
                        if dest is vT:
                            # v2 build interleaved: transpose the 4 ktiles of
                            # this freshly written v token-chunk
                            for kt in range(4 * tcn, 4 * tcn + 4):
                                pt = pst.tile([128, 128], f32r, tag="tp",
                                              name="tr_ps")
                                nc.tensor.transpose(
                                    pt, vT[:, kt * 128:(kt + 1) * 128], ident)
                                base = kt * V2W
                                nc.vector.tensor_copy(
                                    v2[:, base:base + 64], pt[:, 0:64])
                                nc.vector.tensor_copy(
                                    v2[:, base + 65:base + 129],
                                    pt[:, 64:128])

                # ---- attention for this pair, all 4 q-chunks ----
                for qc in range(4):
                    collq = coll_pool.tile([2, 512], f32, tag="cq",
                                           name="collq")
                    collrq = coll_pool.tile([2, 512], f32r, tag="cr",
                                            name="collrq")
                    ngr = 2 * (qc + 1)      # groups of 2 ktiles
                    c_ext = [cxps.tile([65, 512], f32, tag="cext",
                                       name="c_ext") for _ in range(2)]
                    for g in range(ngr):
                        diag = g >= ngr - 2
                        m = g - (ngr - 2)
                        scp = [scps.tile([128, 1024], f32, tag="sc",
                                         name="sc_ps") for _ in range(2)]
                        for j in range(2):
                            kt = 2 * g + j
                            roff = (2 * m + j) * 128 if diag else 0
                            for side in range(2):
                                poff = side * 64
                                nc.tensor.matmul(
                                    scp[side][:, j * 512 + roff:
                                              (j + 1) * 512],
                                    lhsT=kp[poff:poff + 64,
                                            kt * 128:(kt + 1) * 128],
                                    rhs=qp[poff:poff + 64,
                                           qc * 512 + roff:(qc + 1) * 512],
                                    start=True, stop=True)
                        ee = []
                        for side in range(2):
                            e = esb.tile([128, 1024], f32r, tag="e",
                                         name="e_sb")
                            if diag and m == 1:
                                # only ktiles r=2,3 live here; exp just the
                                # valid column ranges
                                nc.scalar.activation(
                                    e[:, 256:512], scp[side][:, 256:512],
                                    AF.Exp, scale=0.125)
                                nc.scalar.activation(
                                    e[:, 896:1024], scp[side][:, 896:1024],
                                    AF.Exp, scale=0.125)
                            else:
                                nc.scalar.activation(e, scp[side], AF.Exp,
                                                     scale=0.125)
                            if diag:
                                # in-tile causal boundary: 128-wide triangle
                                # per diagonal ktile
                                for j in range(2):
                                    r = 2 * m + j
                                    c0 = j * 512 + r * 128
                                    nc.vector.tensor_mul(
                                        e[:, c0:c0 + 128],
                                        e[:, c0:c0 + 128], tri_sb)
                            ee.append(e)
                        for j in range(2):
                            kt = 2 * g + j
                            r = 2 * m + j
                            roff = r * 128 if diag else 0
                            for side in range(2):
                                vb = kt * V2W + side * 65
                                nc.tensor.matmul(
                                    c_ext[side][:, roff:512],
                                    lhsT=v2[:, vb:vb + 65],
                                    rhs=ee[side][:, j * 512 + roff:
                                                 (j + 1) * 512],
                                    start=(g == 0 and j == 0),
                                    stop=(g == ngr - 1 and j == 1))
                    for side in range(2):
                        poff = side * 64
                        # engine APs need 32-aligned partition bases, so the
                        # denominator row (psum partition 64) is staged on
                        # partition 64 and moved to the collector row by DMA
                        dst = stg.tile([65, 512], f32, tag="dstage",
                                       name="dstage")
                        nc.vector.tensor_copy(dst[64:65, :],
                                              c_ext[side][64:65, :])
                        nc.sync.dma_start(out=collq[side:side + 1, :],
                                          in_=dst[64:65, :])
                        nc.vector.tensor_copy(
                            ctx_sb[pair][poff:poff + 64,
                                         qc * 512:(qc + 1) * 512],
                            c_ext[side][0:64, :])
                    # normalize: batched reciprocal of both heads' rows, then
                    # partition-broadcast each row by a step-0 DMA
                    with nc.allow_low_precision(reason="f32r == f32 storage"):
                        nc.vector.reciprocal(collrq, collq)
                    dsc = dscr.tile([2, 512], f32r, tag="ds", name="dsc")
                    nc.scalar.dma_start(out=dsc, in_=collrq)
                    # one [128,512] tile, each head's reciprocal row broadcast
                    # over its own partition half so the multiply's operand
                    # base partitions match
                    bcast = bcq.tile([128, 512], f32r, tag="bc", name="bcast")
                    for side in range(2):
                        nc.scalar.dma_start(
                            out=bcast[side * 64:(side + 1) * 64, :],
                            in_=dsc[side:side + 1, :].to_broadcast(
                                [64, 512]))
                    for side in range(2):
                        poff = side * 64
                        cslice = ctx_sb[pair][poff:poff + 64,
                                              qc * 512:(qc + 1) * 512]
                        nc.vector.tensor_mul(cslice, cslice,
                                             bcast[poff:poff + 64, :])

        # ---------------- tail: out projection ----------------
        with tc.tile_pool(name="wop", bufs=1) as wop, \
             tc.tile_pool(name="yps", bufs=4, space="PSUM") as yps, \
             tc.tile_pool(name="ysbp", bufs=4) as ysbp:
            w_o_sb = [wop.tile([128, C], f32r, tag=f"wo{i}", name=f"wo_sb{i}")
                      for i in range(4)]
            for f in range(4):
                nc.gpsimd.dma_start(out=w_o_sb[f],
                                    in_=w_o[f * 128:(f + 1) * 128, :])
            for tt in range(TK):
                for oc in range(2):
                    yp = yps.tile([128, 512], f32, tag="yp", name="y_ps")
                    for f in range(4):
                        nc.tensor.matmul(
                            yp, lhsT=ctx_sb[f][:, tt * 128:(tt + 1) * 128],
                            rhs=w_o_sb[f][:, oc * 512:(oc + 1) * 512],
                            start=(f == 0), stop=(f == 3))
                    ysb = ysbp.tile([128, 512], f32, tag="ysb", name="y_sb")
                    if oc == 0:
                        nc.scalar.activation(ysb, yp, AF.Copy)
                    else:
                        nc.vector.tensor_copy(ysb, yp)
                    nc.sync.dma_start(
                        out=y_d[tt * 128:(tt + 1) * 128,
                                oc * 512:(oc + 1) * 512],
                        in_=ysb)

    nc.compile()
    return nc


def _host_inputs(x, w_qkv, b_qkv, w_out):
    """Build the 8 per-core input maps."""
    tri = (np.arange(128)[:, None] <= np.arange(128)[None, :]).astype(
        np.float32)

    xt = [np.ascontiguousarray(x[b].T) for b in range(B)]      # [C, T] each
    in_maps = []
    for core in range(NCORES):
        b, hg = core // 2, core % 2
        cs = slice(hg * FQ, (hg + 1) * FQ)
        w_slice = np.concatenate(
            [w_qkv[:, cs], w_qkv[:, C + hg * FQ: C + (hg + 1) * FQ],
             w_qkv[:, 2 * C + hg * FQ: 2 * C + (hg + 1) * FQ]], axis=1)
        b_slice = np.concatenate(
            [b_qkv[cs], b_qkv[C + hg * FQ: C + (hg + 1) * FQ],
             b_qkv[2 * C + hg * FQ: 2 * C + (hg + 1) * FQ]])
        in_maps.append({
            "x_t": xt[b],
            "w_s": np.ascontiguousarray(w_slice),
            "b_s": np.ascontiguousarray(b_slice),
            "w_o": np.ascontiguousarray(w_out[hg * FQ:(hg + 1) * FQ, :]),
            "tri": tri,
            "idm": np.eye(128, dtype=np.float32),
            "onec": np.ones((128, 64), dtype=np.float32),
        })
    return in_maps


def get_program():
    if "nc" not in _CACHE:
        _CACHE["nc"] = _build_program()
    return _CACHE["nc"]


def kernel(x, w_qkv, b_qkv, w_out, b_out):
    from concourse.bass_utils import run_bass_kernel_spmd

    x = np.asarray(x, dtype=np.float32)
    w_qkv = np.asarray(w_qkv, dtype=np.float32)
    b_qkv = np.asarray(b_qkv, dtype=np.float32)
    w_out = np.asarray(w_out, dtype=np.float32)
    b_out = np.asarray(b_out, dtype=np.float32)

    nc = get_program()
    in_maps = _host_inputs(x, w_qkv, b_qkv, w_out)
    res = run_bass_kernel_spmd(nc, in_maps, core_ids=list(range(NCORES)))

    out = np.empty((B, T, C), dtype=np.float32)
    for b in range(B):
        out[b] = res.results[2 * b]["y"] + res.results[2 * b + 1]["y"] + b_out
    return out


# revision 45
# speedup vs baseline: 4.0639x; 1.0010x over previous
"""Causal self-attention (B=4, T=2048, C=1024, H=16, D=64) on 8 trn2 cores.

Sharding: core c handles batch b = c//2 and head-group hg = c%2 (8 heads).
qkv projection is column-parallel, attention is head-parallel, out_proj is
row-parallel; the final 2-way partial-sum + bias happens on host.

Per-core device program, pipelined over head PAIRS so the qkv projection of
pair p+1 overlaps the attention of pair p:
  per pair p (heads 2p, 2p+1, living on partition halves 0-63 / 64-127):
    - qkvT = (W_slice.T @ x.T) + bias -> qT,kT [feat, tok], vT [feat, tok]
    - v2 = PE-transpose(vT) with interleaved ones columns (the ones column
      makes the attn@v matmul also emit the softmax denominator row)
    - per q-chunk: scoresT = kT.T@qT (causal-skipped + sliced), exp on ACT,
      128-wide triangle mask on DVE, ctxT_ext = [v|1].T @ exp in PSUM;
      denominators collected by DMA, batched reciprocal, broadcast across
      partitions by a partition-step-0 DMA, normalize ctx in place
  tail: y_partial = ctx_stacked.T @ W_out_slice -> DRAM
"""

import os
import sys

for _p in ("/opt/trn_rl_repo", "/root/.axon_site/_ro/trn_rl_repo"):
    if os.path.isdir(_p) and _p not in sys.path:
        sys.path.insert(0, _p)

import numpy as np

B, T, C = 4, 2048, 1024
H, D = 16, 64
NCORES = 8
HPC = 8          # heads per core
FQ = HPC * D     # 512 per-core q (=k=v) feature count
TK = T // 128    # 16 token tiles of 128
V2W = 130        # v2 per-ktile width: 64 + 1 + 64 + 1

_CACHE = {}


def _build_program():
    import concourse.bacc as bacc
    import concourse.tile as tile
    import concourse.mybir as mybir
    from contextlib import ExitStack

    f32 = mybir.dt.float32
    f32r = mybir.dt.float32r
    AF = mybir.ActivationFunctionType

    nc = bacc.Bacc("TRN2", target_bir_lowering=False, debug=False)

    x_t = nc.dram_tensor("x_t", [C, T], f32r, kind="ExternalInput").ap()
    w_s = nc.dram_tensor("w_s", [C, 3 * FQ], f32r, kind="ExternalInput").ap()
    b_s = nc.dram_tensor("b_s", [3 * FQ], f32, kind="ExternalInput").ap()
    w_o = nc.dram_tensor("w_o", [FQ, C], f32r, kind="ExternalInput").ap()
    tri_d = nc.dram_tensor("tri", [128, 128], f32, kind="ExternalInput").ap()
    idm_d = nc.dram_tensor("idm", [128, 128], f32r, kind="ExternalInput").ap()
    onec_d = nc.dram_tensor("onec", [128, 64], f32r, kind="ExternalInput").ap()
    y_d = nc.dram_tensor("y", [T, C], f32, kind="ExternalOutput").ap()

    with tile.TileContext(nc) as tc, ExitStack() as ctx:
        # ---- whole-kernel persistents ----
        pp = ctx.enter_context(tc.tile_pool(name="persist", bufs=1))
        ident = pp.tile([128, 128], f32r, tag="ident", name="ident")
        nc.sync.dma_start(out=ident, in_=idm_d)
        b_sb = pp.tile([128, 12], f32, tag="bias", name="b_sb")
        nc.sync.dma_start(out=b_sb, in_=b_s.rearrange("(f p) -> p f", p=128))
        tri_sb = pp.tile([128, 128], f32, tag="tri", name="tri_sb")
        nc.sync.dma_start(out=tri_sb, in_=tri_d)

        ctx_pool = ctx.enter_context(tc.tile_pool(name="ctxp", bufs=1))
        ctx_sb = [ctx_pool.tile([128, T], f32r, tag=f"c{i}", name=f"ctx_sb{i}")
                  for i in range(4)]

        # psum pools that span the whole pair pipeline (8 banks total:
        # qkv 1 + transpose 1 + scores 4 + ctx 2)
        ps1 = ctx.enter_context(tc.tile_pool(name="ps1", bufs=1, space="PSUM"))
        pst = ctx.enter_context(tc.tile_pool(name="pst", bufs=1, space="PSUM"))

        with tc.tile_pool(name="xres", bufs=1) as xp, \
             tc.tile_pool(name="wstr", bufs=2) as wp, \
             tc.tile_pool(name="qkq", bufs=2) as qkq, \
             tc.tile_pool(name="vTq", bufs=2) as vTq, \
             tc.tile_pool(name="v2q", bufs=2) as v2q, \
             tc.tile_pool(name="collp", bufs=3) as coll_pool, \
             tc.tile_pool(name="bcq", bufs=3) as bcq, \
             tc.tile_pool(name="dscr", bufs=4, space="DRAM") as dscr, \
             tc.tile_pool(name="stg", bufs=4) as stg, \
             tc.tile_pool(name="esb", bufs=3) as esb, \
             tc.tile_pool(name="scps", bufs=2, space="PSUM") as scps, \
             tc.tile_pool(name="cxps", bufs=2, space="PSUM") as cxps:

            x_sb = [xp.tile([128, T], f32r, tag=f"x{ks}", name=f"x_sb{ks}")
                    for ks in range(8)]
            # chunked tcn-outer so the first token-chunk's 8 k-slices arrive
            # quickly and the first psum accumulation can start early
            for tcn in range(4):
                for ks in range(8):
                    nc.sync.dma_start(
                        out=x_sb[ks][:, tcn * 512:(tcn + 1) * 512],
                        in_=x_t[ks * 128:(ks + 1) * 128,
                                tcn * 512:(tcn + 1) * 512])

            for pair in range(4):
                # ---- qkv projection for this pair (v first, then q, k) ----
                qp = qkq.tile([128, T], f32r, tag="qp", name="q_p")
                kp = qkq.tile([128, T], f32r, tag="kp", name="k_p")
                vT = vTq.tile([128, T], f32r, tag="vT", name="vT_p")
                v2 = v2q.tile([128, TK * V2W], f32r, tag="v2", name="v2_p")
                v2v = v2.rearrange("p (t w) -> p t w", w=V2W)
                onec3 = onec_d[:, 0:16].rearrange("p (t o) -> p t o", o=1)
                nc.sync.dma_start(out=v2v[:, :, 64:65], in_=onec3)
                nc.sync.dma_start(out=v2v[:, :, 129:130], in_=onec3)

                for ft, dest in ((8 + pair, vT), (pair, qp), (4 + pair, kp)):
                    wt = [wp.tile([128, 128], f32r, tag=f"w{ks}",
                                  name=f"wt{ks}") for ks in range(8)]
                    for ks in range(8):
                        nc.gpsimd.dma_start(
                            out=wt[ks],
                            in_=w_s[ks * 128:(ks + 1) * 128,
                                    ft * 128:(ft + 1) * 128])
                    for tcn in range(4):
                        ps = ps1.tile([128, 512], f32, tag="qkvps",
                                      name="qkv_ps")
                        for ks in range(8):
                            nc.tensor.matmul(
                                ps, lhsT=wt[ks],
                                rhs=x_sb[ks][:, tcn * 512:(tcn + 1) * 512],
                                start=(ks == 0), stop=(ks == 7))
                        # evacuate with fused bias add: out = psum + b
                        nc.scalar.activation(
                            dest[:, tcn * 512:(tcn + 1) * 512], ps,
                            AF.Identity, bias=b_sb[:, ft:ft + 1], scale=1.0)
                        if dest is vT:
                            # v2 build interleaved: transpose the 4 ktiles of
                            # this freshly written v token-chunk
                            for kt in range(4 * tcn, 4 * tcn + 4):
                                pt = pst.tile([128, 128], f32r, tag="tp",
                                              name="tr_ps")
                                nc.tensor.transpose(
                                    pt, vT[:, kt * 128:(kt + 1) * 128], ident)
                                base = kt * V2W
                                nc.vector.tensor_copy(
                                    v2[:, base:base + 64], pt[:, 0:64])
                                nc.vector.tensor_copy(
                                    v2[:, base + 65:base + 129],
                                    pt[:, 64:128])

                # ---- attention for this pair, all 4 q-chunks ----
                for qc in range(4):
                    collq = coll_pool.tile([2, 512], f32, tag="cq",
                                           name="collq")
                    collrq = coll_pool.tile([2, 512], f32r, tag="cr",
                                            name="collrq")
                    ngr = 2 * (qc + 1)      # groups of 2 ktiles
                    c_ext = [cxps.tile([65, 512], f32, tag="cext",
                                       name="c_ext") for _ in range(2)]
                    for g in range(ngr):
                        diag = g >= ngr - 2
                        m = g - (ngr - 2)
                        scp = [scps.tile([128, 1024], f32, tag="sc",
                                         name="sc_ps") for _ in range(2)]
                        for j in range(2):
                            kt = 2 * g + j
                            roff = (2 * m + j) * 128 if diag else 0
                            for side in range(2):
                                poff = side * 64
                                nc.tensor.matmul(
                                    scp[side][:, j * 512 + roff:
                                              (j + 1) * 512],
                                    lhsT=kp[poff:poff + 64,
                                            kt * 128:(kt + 1) * 128],
                                    rhs=qp[poff:poff + 64,
                                           qc * 512 + roff:(qc + 1) * 512],
                                    start=True, stop=True)
                        ee = []
                        for side in range(2):
                            e = esb.tile([128, 1024], f32r, tag="e",
                                         name="e_sb")
                            if diag and m == 1:
                                # only ktiles r=2,3 live here; exp just the
                                # valid column ranges
                                nc.scalar.activation(
                                    e[:, 256:512], scp[side][:, 256:512],
                                    AF.Exp, scale=0.125)
                                nc.scalar.activation(
                                    e[:, 896:1024], scp[side][:, 896:1024],
                                    AF.Exp, scale=0.125)
                            else:
                                nc.scalar.activation(e, scp[side], AF.Exp,
                                                     scale=0.125)
                            if diag:
                                # in-tile causal boundary: 128-wide triangle
                                # per diagonal ktile
                                for j in range(2):
                                    r = 2 * m + j
                                    c0 = j * 512 + r * 128
                                    nc.vector.tensor_mul(
                                        e[:, c0:c0 + 128],
                                        e[:, c0:c0 + 128], tri_sb)
                            ee.append(e)
                        for j in range(2):
                            kt = 2 * g + j
                            r = 2 * m + j
                            roff = r * 128 if diag else 0
                            for side in range(2):
                                vb = kt * V2W + side * 65
                                nc.tensor.matmul(
                                    c_ext[side][:, roff:512],
                                    lhsT=v2[:, vb:vb + 65],
                                    rhs=ee[side][:, j * 512 + roff:
                                                 (j + 1) * 512],
                                    start=(g == 0 and j == 0),
                                    stop=(g == ngr - 1 and j == 1))
                    for side in range(2):
                        poff = side * 64
                        # engine APs need 32-aligned partition bases, so the
                        # denominator row (psum partition 64) is staged on
                        # partition 64 and moved to the collector row by DMA
                        dst = stg.tile([65, 512], f32, tag="dstage",
                                       name="dstage")
                        nc.vector.tensor_copy(dst[64:65, :],
                                              c_ext[side][64:65, :])
                        nc.sync.dma_start(out=collq[side:side + 1, :],
                                          in_=dst[64:65, :])
                        nc.vector.tensor_copy(
                            ctx_sb[pair][poff:poff + 64,
                                         qc * 512:(qc + 1) * 512],
                            c_ext[side][0:64, :])
                    # normalize: batched reciprocal of both heads' rows, then
                    # partition-broadcast each row by a step-0 DMA
                    with nc.allow_low_precision(reason="f32r == f32 storage"):
                        nc.vector.reciprocal(collrq, collq)
                    dsc = dscr.tile([2, 512], f32r, tag="ds", name="dsc")
                    nc.scalar.dma_start(out=dsc, in_=collrq)
                    # one [128,512] tile, each head's reciprocal row broadcast
                    # over its own partition half so the multiply's operand
                    # base partitions match
                    bcast = bcq.tile([128, 512], f32r, tag="bc", name="bcast")
                    for side in range(2):
                        nc.scalar.dma_start(
                            out=bcast[side * 64:(side + 1) * 64, :],
                            in_=dsc[side:side + 1, :].to_broadcast(
                                [64, 512]))
                    for side in range(2):
                        poff = side * 64
                        cslice = ctx_sb[pair][poff:poff + 64,
                                              qc * 512:(qc + 1) * 512]
                        nc.vector.tensor_mul(cslice, cslice,
                                             bcast[poff:poff + 64, :])

        # ---------------- tail: out projection ----------------
        with tc.tile_pool(name="wop", bufs=1) as wop, \
             tc.tile_pool(name="yps", bufs=4, space="PSUM") as yps, \
             tc.tile_pool(name="ysbp", bufs=4) as ysbp:
            w_o_sb = [wop.tile([128, C], f32r, tag=f"wo{i}", name=f"wo_sb{i}")
                      for i in range(4)]
            for f in range(4):
                nc.gpsimd.dma_start(out=w_o_sb[f],
                                    in_=w_o[f * 128:(f + 1) * 128, :])
            for tt in range(TK):
                for oc in range(2):
                    yp = yps.tile([128, 512], f32, tag="yp", name="y_ps")
                    for f in range(4):
                        nc.tensor.matmul(
                            yp, lhsT=ctx_sb[f][:, tt * 128:(tt + 1) * 128],
                            rhs=w_o_sb[f][:, oc * 512:(oc + 1) * 512],
                            start=(f == 0), stop=(f == 3))
                    ysb = ysbp.tile([128, 512], f32, tag="ysb", name="y_sb")
                    if oc == 0:
                        nc.scalar.activation(ysb, yp, AF.Copy)
                    else:
                        nc.vector.tensor_copy(ysb, yp)
                    nc.sync.dma_start(
                        out=y_d[tt * 128:(tt + 1) * 128,
                                oc * 512:(oc + 1) * 512],
                        in_=ysb)

    nc.compile()
    return nc


def _host_inputs(x, w_qkv, b_qkv, w_out):
    """Build the 8 per-core input maps."""
    tri = (np.arange(128)[:, None] <= np.arange(128)[None, :]).astype(
        np.float32)

    xt = [np.ascontiguousarray(x[b].T) for b in range(B)]      # [C, T] each
    in_maps = []
    for core in range(NCORES):
        b, hg = core // 2, core % 2
        cs = slice(hg * FQ, (hg + 1) * FQ)
        w_slice = np.concatenate(
            [w_qkv[:, cs], w_qkv[:, C + hg * FQ: C + (hg + 1) * FQ],
             w_qkv[:, 2 * C + hg * FQ: 2 * C + (hg + 1) * FQ]], axis=1)
        b_slice = np.concatenate(
            [b_qkv[cs], b_qkv[C + hg * FQ: C + (hg + 1) * FQ],
             b_qkv[2 * C + hg * FQ: 2 * C + (hg + 1) * FQ]])
        in_maps.append({
            "x_t": xt[b],
            "w_s": np.ascontiguousarray(w_slice),
            "b_s": np.ascontiguousarray(b_slice),
            "w_o": np.ascontiguousarray(w_out[hg * FQ:(hg + 1) * FQ, :]),
            "tri": tri,
            "idm": np.eye(128, dtype=np.float32),
            "onec": np.ones((128, 64), dtype=np.float32),
        })
    return in_maps


def get_program():
    if "nc" not in _CACHE:
        _CACHE["nc"] = _build_program()
    return _CACHE["nc"]


def kernel(x, w_qkv, b_qkv, w_out, b_out):
    from concourse.bass_utils import run_bass_kernel_spmd

    x = np.asarray(x, dtype=np.float32)
    w_qkv = np.asarray(w_qkv, dtype=np.float32)
    b_qkv = np.asarray(b_qkv, dtype=np.float32)
    w_out = np.asarray(w_out, dtype=np.float32)
    b_out = np.asarray(b_out, dtype=np.float32)

    nc = get_program()
    in_maps = _host_inputs(x, w_qkv, b_qkv, w_out)
    res = run_bass_kernel_spmd(nc, in_maps, core_ids=list(range(NCORES)))

    out = np.empty((B, T, C), dtype=np.float32)
    for b in range(B):
        out[b] = res.results[2 * b]["y"] + res.results[2 * b + 1]["y"] + b_out
    return out


# revision 48
# speedup vs baseline: 4.2299x; 1.0409x over previous
"""Causal self-attention (B=4, T=2048, C=1024, H=16, D=64) on 8 trn2 cores.

Sharding: core c handles batch b = c//2 and head-group hg = c%2 (8 heads).
qkv projection is column-parallel, attention is head-parallel, out_proj is
row-parallel; the final 2-way partial-sum + bias happens on host.

Per-core device program, pipelined over head PAIRS so the qkv projection of
pair p+1 overlaps the attention of pair p:
  per pair p (heads 2p, 2p+1, living on partition halves 0-63 / 64-127):
    - qkvT = (W_slice.T @ x.T) + bias -> qT,kT [feat, tok], vT [feat, tok]
    - v2 = PE-transpose(vT) with interleaved ones columns (the ones column
      makes the attn@v matmul also emit the softmax denominator row)
    - per q-chunk: scoresT = kT.T@qT (causal-skipped + sliced), exp on ACT,
      128-wide triangle mask on DVE, ctxT_ext = [v|1].T @ exp in PSUM;
      denominators collected by DMA, batched reciprocal, broadcast across
      partitions by a partition-step-0 DMA, normalize ctx in place
  tail: y_partial = ctx_stacked.T @ W_out_slice -> DRAM
"""

import os
import sys

for _p in ("/opt/trn_rl_repo", "/root/.axon_site/_ro/trn_rl_repo"):
    if os.path.isdir(_p) and _p not in sys.path:
        sys.path.insert(0, _p)

import numpy as np

B, T, C = 4, 2048, 1024
H, D = 16, 64
NCORES = 8
HPC = 8          # heads per core
FQ = HPC * D     # 512 per-core q (=k=v) feature count
TK = T // 128    # 16 token tiles of 128
V2W = 130        # v2 per-ktile width: 64 + 1 + 64 + 1

_CACHE = {}


def _build_program():
    import concourse.bacc as bacc
    import concourse.tile as tile
    import concourse.mybir as mybir
    from contextlib import ExitStack

    f32 = mybir.dt.float32
    f32r = mybir.dt.float32r
    AF = mybir.ActivationFunctionType

    nc = bacc.Bacc("TRN2", target_bir_lowering=False, debug=False)

    x_t = nc.dram_tensor("x_t", [C, T], f32r, kind="ExternalInput").ap()
    w_s = nc.dram_tensor("w_s", [C, 3 * FQ], f32r, kind="ExternalInput").ap()
    b_s = nc.dram_tensor("b_s", [3 * FQ], f32, kind="ExternalInput").ap()
    w_o = nc.dram_tensor("w_o", [FQ, C], f32r, kind="ExternalInput").ap()
    tri_d = nc.dram_tensor("tri", [128, 128], f32, kind="ExternalInput").ap()
    idm_d = nc.dram_tensor("idm", [128, 128], f32r, kind="ExternalInput").ap()
    onec_d = nc.dram_tensor("onec", [128, 64], f32r, kind="ExternalInput").ap()
    y_d = nc.dram_tensor("y", [T, C], f32, kind="ExternalOutput").ap()

    with tile.TileContext(nc) as tc, ExitStack() as ctx:
        # ---- whole-kernel persistents ----
        pp = ctx.enter_context(tc.tile_pool(name="persist", bufs=1))
        ident = pp.tile([128, 128], f32r, tag="ident", name="ident")
        nc.sync.dma_start(out=ident, in_=idm_d)
        b_sb = pp.tile([128, 12], f32, tag="bias", name="b_sb")
        nc.sync.dma_start(out=b_sb, in_=b_s.rearrange("(f p) -> p f", p=128))
        tri_sb = pp.tile([128, 128], f32, tag="tri", name="tri_sb")
        nc.sync.dma_start(out=tri_sb, in_=tri_d)

        ctx_pool = ctx.enter_context(tc.tile_pool(name="ctxp", bufs=1))
        ctx_sb = [ctx_pool.tile([128, T], f32r, tag=f"c{i}", name=f"ctx_sb{i}")
                  for i in range(4)]

        # psum pools that span the whole pair pipeline (8 banks total:
        # qkv 1 + transpose 1 + scores 4 + ctx 2)
        ps1 = ctx.enter_context(tc.tile_pool(name="ps1", bufs=1, space="PSUM"))
        pst = ctx.enter_context(tc.tile_pool(name="pst", bufs=1, space="PSUM"))

        with tc.tile_pool(name="xres", bufs=1) as xp, \
             tc.tile_pool(name="wstr", bufs=2) as wp, \
             tc.tile_pool(name="qkq", bufs=2) as qkq, \
             tc.tile_pool(name="vTq", bufs=2) as vTq, \
             tc.tile_pool(name="v2q", bufs=2) as v2q, \
             tc.tile_pool(name="collp", bufs=3) as coll_pool, \
             tc.tile_pool(name="bcq", bufs=3) as bcq, \
             tc.tile_pool(name="dscr", bufs=4, space="DRAM") as dscr, \
             tc.tile_pool(name="stg", bufs=4) as stg, \
             tc.tile_pool(name="esb", bufs=3) as esb, \
             tc.tile_pool(name="scps", bufs=2, space="PSUM") as scps, \
             tc.tile_pool(name="cxps", bufs=2, space="PSUM") as cxps:

            x_sb = [xp.tile([128, T], f32r, tag=f"x{ks}", name=f"x_sb{ks}")
                    for ks in range(8)]
            # chunked tcn-outer so the first token-chunk's 8 k-slices arrive
            # quickly and the first psum accumulation can start early
            for tcn in range(4):
                for ks in range(8):
                    nc.sync.dma_start(
                        out=x_sb[ks][:, tcn * 512:(tcn + 1) * 512],
                        in_=x_t[ks * 128:(ks + 1) * 128,
                                tcn * 512:(tcn + 1) * 512])

            for pair in range(4):
                # ---- qkv projection for this pair (v first, then q, k) ----
                qp = qkq.tile([128, T], f32r, tag="qp", name="q_p")
                kp = qkq.tile([128, T], f32r, tag="kp", name="k_p")
                vT = vTq.tile([128, T], f32r, tag="vT", name="vT_p")
                v2 = v2q.tile([128, TK * V2W], f32r, tag="v2", name="v2_p")
                v2v = v2.rearrange("p (t w) -> p t w", w=V2W)
                onec3 = onec_d[:, 0:16].rearrange("p (t o) -> p t o", o=1)
                nc.sync.dma_start(out=v2v[:, :, 64:65], in_=onec3)
                nc.sync.dma_start(out=v2v[:, :, 129:130], in_=onec3)

                for ft, dest in ((8 + pair, vT), (pair, qp), (4 + pair, kp)):
                    wt = [wp.tile([128, 128], f32r, tag=f"w{ks}",
                                  name=f"wt{ks}") for ks in range(8)]
                    for ks in range(8):
                        nc.gpsimd.dma_start(
                            out=wt[ks],
                            in_=w_s[ks * 128:(ks + 1) * 128,
                                    ft * 128:(ft + 1) * 128])
                    for tcn in range(4):
                        # double-buffer the qkv psum by ping-ponging between
                        # the two single-slot pools; the transpose slot is
                        # only contended during the v third
                        if dest is vT or tcn % 2 == 0:
                            ps = ps1.tile([128, 512], f32, tag="qkvps",
                                          name="qkv_ps")
                        else:
                            ps = pst.tile([128, 512], f32, tag="tp",
                                          name="qkv_ps2")
                        for ks in range(8):
                            nc.tensor.matmul(
                                ps, lhsT=wt[ks],
                                rhs=x_sb[ks][:, tcn * 512:(tcn + 1) * 512],
                                start=(ks == 0), stop=(ks == 7))
                        # evacuate with fused bias add: out = psum + b
                        nc.scalar.activation(
                            dest[:, tcn * 512:(tcn + 1) * 512], ps,
                            AF.Identity, bias=b_sb[:, ft:ft + 1], scale=1.0)
                        if dest is vT:
                            # v2 build interleaved: transpose the 4 ktiles of
                            # this freshly written v token-chunk
                            for kt in range(4 * tcn, 4 * tcn + 4):
                                pt = pst.tile([128, 128], f32r, tag="tp",
                                              name="tr_ps")
                                nc.tensor.transpose(
                                    pt, vT[:, kt * 128:(kt + 1) * 128], ident)
                                base = kt * V2W
                                nc.vector.tensor_copy(
                                    v2[:, base:base + 64], pt[:, 0:64])
                                nc.vector.tensor_copy(
                                    v2[:, base + 65:base + 129],
                                    pt[:, 64:128])

                # ---- attention for this pair, all 4 q-chunks ----
                for qc in range(4):
                    collq = coll_pool.tile([2, 512], f32, tag="cq",
                                           name="collq")
                    collrq = coll_pool.tile([2, 512], f32r, tag="cr",
                                            name="collrq")
                    ngr = 2 * (qc + 1)      # groups of 2 ktiles
                    c_ext = [cxps.tile([65, 512], f32, tag="cext",
                                       name="c_ext") for _ in range(2)]
                    for g in range(ngr):
                        diag = g >= ngr - 2
                        m = g - (ngr - 2)
                        scp = [scps.tile([128, 1024], f32, tag="sc",
                                         name="sc_ps") for _ in range(2)]
                        for j in range(2):
                            kt = 2 * g + j
                            roff = (2 * m + j) * 128 if diag else 0
                            for side in range(2):
                                poff = side * 64
                                nc.tensor.matmul(
                                    scp[side][:, j * 512 + roff:
                                              (j + 1) * 512],
                                    lhsT=kp[poff:poff + 64,
                                            kt * 128:(kt + 1) * 128],
                                    rhs=qp[poff:poff + 64,
                                           qc * 512 + roff:(qc + 1) * 512],
                                    start=True, stop=True)
                        ee = []
                        for side in range(2):
                            e = esb.tile([128, 1024], f32r, tag="e",
                                         name="e_sb")
                            if diag and m == 1:
                                # only ktiles r=2,3 live here; exp just the
                                # valid column ranges
                                nc.scalar.activation(
                                    e[:, 256:512], scp[side][:, 256:512],
                                    AF.Exp, scale=0.125)
                                nc.scalar.activation(
                                    e[:, 896:1024], scp[side][:, 896:1024],
                                    AF.Exp, scale=0.125)
                            else:
                                nc.scalar.activation(e, scp[side], AF.Exp,
                                                     scale=0.125)
                            if diag:
                                # in-tile causal boundary: 128-wide triangle
                                # per diagonal ktile
                                for j in range(2):
                                    r = 2 * m + j
                                    c0 = j * 512 + r * 128
                                    nc.vector.tensor_mul(
                                        e[:, c0:c0 + 128],
                                        e[:, c0:c0 + 128], tri_sb)
                            ee.append(e)
                        for j in range(2):
                            kt = 2 * g + j
                            r = 2 * m + j
                            roff = r * 128 if diag else 0
                            for side in range(2):
                                vb = kt * V2W + side * 65
                                nc.tensor.matmul(
                                    c_ext[side][:, roff:512],
                                    lhsT=v2[:, vb:vb + 65],
                                    rhs=ee[side][:, j * 512 + roff:
                                                 (j + 1) * 512],
                                    start=(g == 0 and j == 0),
                                    stop=(g == ngr - 1 and j == 1))
                    for side in range(2):
                        poff = side * 64
                        # engine APs need 32-aligned partition bases, so the
                        # denominator row (psum partition 64) is staged on
                        # partition 64 and moved to the collector row by DMA
                        dst = stg.tile([65, 512], f32, tag="dstage",
                                       name="dstage")
                        nc.vector.tensor_copy(dst[64:65, :],
                                              c_ext[side][64:65, :])
                        nc.sync.dma_start(out=collq[side:side + 1, :],
                                          in_=dst[64:65, :])
                        nc.vector.tensor_copy(
                            ctx_sb[pair][poff:poff + 64,
                                         qc * 512:(qc + 1) * 512],
                            c_ext[side][0:64, :])
                    # normalize: batched reciprocal of both heads' rows, then
                    # partition-broadcast each row by a step-0 DMA
                    with nc.allow_low_precision(reason="f32r == f32 storage"):
                        nc.vector.reciprocal(collrq, collq)
                    dsc = dscr.tile([2, 512], f32r, tag="ds", name="dsc")
                    nc.scalar.dma_start(out=dsc, in_=collrq)
                    # one [128,512] tile, each head's reciprocal row broadcast
                    # over its own partition half so the multiply's operand
                    # base partitions match
                    bcast = bcq.tile([128, 512], f32r, tag="bc", name="bcast")
                    for side in range(2):
                        nc.scalar.dma_start(
                            out=bcast[side * 64:(side + 1) * 64, :],
                            in_=dsc[side:side + 1, :].to_broadcast(
                                [64, 512]))
                    for side in range(2):
                        poff = side * 64
                        cslice = ctx_sb[pair][poff:poff + 64,
                                              qc * 512:(qc + 1) * 512]
                        nc.vector.tensor_mul(cslice, cslice,
                                             bcast[poff:poff + 64, :])

        # ---------------- tail: out projection ----------------
        with tc.tile_pool(name="wop", bufs=1) as wop, \
             tc.tile_pool(name="yps", bufs=4, space="PSUM") as yps, \
             tc.tile_pool(name="ysbp", bufs=4) as ysbp:
            w_o_sb = [wop.tile([128, C], f32r, tag=f"wo{i}", name=f"wo_sb{i}")
                      for i in range(4)]
            for f in range(4):
                nc.gpsimd.dma_start(out=w_o_sb[f],
                                    in_=w_o[f * 128:(f + 1) * 128, :])
            for tt in range(TK):
                for oc in range(2):
                    yp = yps.tile([128, 512], f32, tag="yp", name="y_ps")
                    for f in range(4):
                        nc.tensor.matmul(
                            yp, lhsT=ctx_sb[f][:, tt * 128:(tt + 1) * 128],
                            rhs=w_o_sb[f][:, oc * 512:(oc + 1) * 512],
                            start=(f == 0), stop=(f == 3))
                    ysb = ysbp.tile([128, 512], f32, tag="ysb", name="y_sb")
                    if oc == 0:
                        nc.scalar.activation(ysb, yp, AF.Copy)
                    else:
                        nc.vector.tensor_copy(ysb, yp)
                    nc.sync.dma_start(
                        out=y_d[tt * 128:(tt + 1) * 128,
                                oc * 512:(oc + 1) * 512],
                        in_=ysb)

    nc.compile()
    return nc


def _host_inputs(x, w_qkv, b_qkv, w_out):
    """Build the 8 per-core input maps."""
    tri = (np.arange(128)[:, None] <= np.arange(128)[None, :]).astype(
        np.float32)

    xt = [np.ascontiguousarray(x[b].T) for b in range(B)]      # [C, T] each
    in_maps = []
    for core in range(NCORES):
        b, hg = core // 2, core % 2
        cs = slice(hg * FQ, (hg + 1) * FQ)
        w_slice = np.concatenate(
            [w_qkv[:, cs], w_qkv[:, C + hg * FQ: C + (hg + 1) * FQ],
             w_qkv[:, 2 * C + hg * FQ: 2 * C + (hg + 1) * FQ]], axis=1)
        b_slice = np.concatenate(
            [b_qkv[cs], b_qkv[C + hg * FQ: C + (hg + 1) * FQ],
             b_qkv[2 * C + hg * FQ: 2 * C + (hg + 1) * FQ]])
        in_maps.append({
            "x_t": xt[b],
            "w_s": np.ascontiguousarray(w_slice),
            "b_s": np.ascontiguousarray(b_slice),
            "w_o": np.ascontiguousarray(w_out[hg * FQ:(hg + 1) * FQ, :]),
            "tri": tri,
            "idm": np.eye(128, dtype=np.float32),
            "onec": np.ones((128, 64), dtype=np.float32),
        })
    return in_maps


def get_program():
    if "nc" not in _CACHE:
        _CACHE["nc"] = _build_program()
    return _CACHE["nc"]


def kernel(x, w_qkv, b_qkv, w_out, b_out):
    from concourse.bass_utils import run_bass_kernel_spmd

    x = np.asarray(x, dtype=np.float32)
    w_qkv = np.asarray(w_qkv, dtype=np.float32)
    b_qkv = np.asarray(b_qkv, dtype=np.float32)
    w_out = np.asarray(w_out, dtype=np.float32)
    b_out = np.asarray(b_out, dtype=np.float32)

    nc = get_program()
    in_maps = _host_inputs(x, w_qkv, b_qkv, w_out)
    res = run_bass_kernel_spmd(nc, in_maps, core_ids=list(range(NCORES)))

    out = np.empty((B, T, C), dtype=np.float32)
    for b in range(B):
        out[b] = res.results[2 * b]["y"] + res.results[2 * b + 1]["y"] + b_out
    return out
